# revision 28
# baseline (speedup 1.0000x reference)
import sys
import hashlib
from types import SimpleNamespace
from concurrent.futures import ThreadPoolExecutor

import numpy as np

sys.path.insert(0, "/opt/trn_rl_repo")

B, N, M = 8, 2048, 256
NCORES = 8
U = 64  # unroll factor inside hardware loops

# Output encoding:
#  "tri": permute probs columns by selection order on-device -> masked zeros
#         become a static lower triangle; ship only the packed upper triangle
#         (u8-quantized) + the rank vector, reconstruct on the host.
#  "u8":  full [N,N] probs matrix quantized to uint8.
#  probs are in [0,1]; code = round(p*OUT_SCALE) on the Act engine (round-to-
#  nearest cast), decode v/OUT_SCALE on host: quant err <= 0.5/253 ~ 2e-3.
OUT_MODE = "tri"  # "tri" | "u8" | "f16" | "f32"
OUT_SCALE = 253.0
# Pipeline groups: cores are split into GROUPS sets run as separate staggered
# dispatches so group A's exec+download overlaps group B's upload on the
# half-duplex tunnel. GROUPS=1 is the plain single-dispatch path.
GROUPS = 2
# Block-packed triangle: each 32-row band [t0, t0+32) keeps columns [t0, N) —
# the static lower-triangle rest is exactly zero and is dropped. 2.13MB/core
# vs 4.2MB dense, with 64 rectangular DMAs per core.
RB = 32  # band height
NB = N // RB  # 64 bands
BW = [N - i * RB for i in range(NB)]  # band widths
CO = [sum(BW[:i]) for i in range(NB)]  # band column offsets in packed layout
CW = sum(BW)  # packed columns per 32-row band layer

_cache = {}


def _build():
    if "nc" in _cache:
        return _cache["nc"]
    from concourse import bass, tile, bacc
    import concourse.mybir as mybir

    fp32 = mybir.dt.float32
    u8 = mybir.dt.uint8
    odt = {"tri": u8, "u8": u8, "f16": mybir.dt.float16,
           "f32": fp32}[OUT_MODE]
    Alu = mybir.AluOpType
    Act = mybir.ActivationFunctionType
    AX = mybir.AxisListType
    ds = bass.ds

    nc = bacc.Bacc("TRN2", target_bir_lowering=False, debug=False,
                   num_devices=NCORES)

    node_a = nc.dram_tensor("node_a", [N // 2, M], fp32, kind="ExternalInput").ap()
    node_b = nc.dram_tensor("node_b", [N // 2, M], fp32, kind="ExternalInput").ap()
    c0_d = nc.dram_tensor("c0", [128, 2], fp32, kind="ExternalInput").ap()
    h0_d = nc.dram_tensor("h0", [128, 2], fp32, kind="ExternalInput").ap()
    xb_d = nc.dram_tensor("xb", [128, 8], fp32, kind="ExternalInput").ap()
    wstat_d = nc.dram_tensor("wstat", [128, 2048], fp32, kind="ExternalInput").ap()
    wkt_d = nc.dram_tensor("wkt", [128, 512], fp32, kind="ExternalInput").ap()
    wqt_d = nc.dram_tensor("wqt", [128, 512], fp32, kind="ExternalInput").ap()
    bkt_d = nc.dram_tensor("bkt", [128, 2], fp32, kind="ExternalInput").ap()
    bqt_d = nc.dram_tensor("bqt", [128, 2], fp32, kind="ExternalInput").ap()
    ones_d = nc.dram_tensor("ones1", [1, 128], fp32, kind="ExternalInput").ap()
    id_d = nc.dram_tensor("ident", [128, 128], fp32, kind="ExternalInput").ap()
    tvs_d = nc.dram_tensor("tvs", [128, 16], fp32, kind="ExternalInput").ap()
    if OUT_MODE == "tri":
        iota1_d = nc.dram_tensor("iota1", [1, N], fp32, kind="ExternalInput").ap()
        outp_d = nc.dram_tensor("outp", [RB, CW], u8, kind="ExternalOutput").ap()
        outr_d = nc.dram_tensor("outr", [128, 16], fp32, kind="ExternalOutput").ap()
        out_d = None
    else:
        out_d = nc.dram_tensor("out", [N, N], odt, kind="ExternalOutput").ap()

    def emit_pass(tc, rep):
        sx = f"_r{rep}"
        # ---- constants in SBUF ----
        xb_sb, f_xb = tc.tile([128, 8], fp32, name="xb_sb" + sx)
        wstat_sb, f_wstat = tc.tile([128, 2048], fp32, name="wstat_sb" + sx)
        wkt_sb, f_wkt = tc.tile([128, 512], fp32, name="wkt_sb" + sx)
        wqt_sb, f_wqt = tc.tile([128, 512], fp32, name="wqt_sb" + sx)
        bkt_sb, f_bkt = tc.tile([128, 2], fp32, name="bkt_sb" + sx)
        bqt_sb, f_bqt = tc.tile([128, 2], fp32, name="bqt_sb" + sx)
        ones_sb, f_ones = tc.tile([1, 128], fp32, name="ones_sb" + sx)
        id_sb, f_id = tc.tile([128, 128], fp32, name="id_sb" + sx)
        tvs_sb, f_tvs = tc.tile([128, 16], fp32, name="tvs_sb" + sx)
        for sb, dr in ((xb_sb, xb_d), (wstat_sb, wstat_d), (wkt_sb, wkt_d),
                       (wqt_sb, wqt_d), (bkt_sb, bkt_d), (bqt_sb, bqt_d),
                       (ones_sb, ones_d), (id_sb, id_d), (tvs_sb, tvs_d)):
            nc.gpsimd.dma_start(sb[:], dr[:, :])

        # main PSUM pool used across all phases
        p512_cm = tc.tile_pool(name="p512" + sx, bufs=2, space="PSUM")
        p512 = p512_cm.__enter__()

        # ---- persistent big tensors ----
        # Alloc order is load-bearing: the tile allocator releases strictly
        # LIFO, so tiles that die earliest are allocated last.
        QT, f_QT = tc.tile([128, 2, N], fp32, name="QT" + sx)
        if OUT_MODE == "tri":
            KpT, f_KpT = tc.tile([128, 2, N], fp32, name="KpT" + sx)
            iotab, f_iotab = tc.tile([128, N], fp32, name="iotab" + sx)
            iota_sb, f_iota = tc.tile([1, N], fp32, name="iota_sb" + sx)
            keysN, f_keysN = tc.tile([128, 16, 256], fp32, name="keysN" + sx)
        keysT, f_keysT = tc.tile([128, 2, N], fp32, name="keysT" + sx)

        # ---- prologue: node -> nodeT -> keysT (+ keysN for tri) ----
        nodeN, f_nodeN = tc.tile([128, 16, 256], fp32, name="nodeN" + sx)
        nodeT, f_nodeT = tc.tile([128, 2, N], fp32, name="nodeT" + sx)
        for c in range(16):
            src = node_a if c < 8 else node_b
            c0_ = c if c < 8 else c - 8
            nc.gpsimd.dma_start(nodeN[:, c, :], src[c0_ * 128:(c0_ + 1) * 128, :])
        ptp_cm = tc.tile_pool(name="ptp" + sx, bufs=2, space="PSUM")
        ptp = ptp_cm.__enter__()
        for c in range(16):
            for k in range(2):
                pt = ptp.tile([128, 128], fp32)
                nc.tensor.transpose(pt[:], nodeN[:, c, k * 128:(k + 1) * 128], id_sb[:])
                nc.scalar.activation(nodeT[:, k, c * 128:(c + 1) * 128], pt[:], Act.Copy)
        ptp_cm.__exit__(None, None, None)
        for j2 in range(2):
            for nb in range(4):
                pk = p512.tile([128, 512], fp32, tag="ps")
                for k in range(2):
                    nc.tensor.matmul(pk[:], wkt_sb[:, (k * 2 + j2) * 128:(k * 2 + j2 + 1) * 128],
                                     nodeT[:, k, nb * 512:(nb + 1) * 512],
                                     start=(k == 0), stop=(k == 1))
                nc.vector.tensor_scalar(out=keysT[:, j2, nb * 512:(nb + 1) * 512],
                                        in0=pk[:], scalar1=bkt_sb[:, j2:j2 + 1],
                                        scalar2=None, op0=Alu.add)
        if OUT_MODE == "tri":
            # keysN[n(p), m(f)] = (node @ Wk^T), unbiased, n-on-partition.
            # Used later to gather keys by selection order via a one-hot matmul.
            for c in range(16):
                pn = p512.tile([128, 256], fp32, tag="ps")
                for kc in range(2):
                    nc.tensor.matmul(pn[:], nodeT[:, kc, c * 128:(c + 1) * 128],
                                     wkt_sb[:, kc * 256:(kc + 1) * 256],
                                     start=(kc == 0), stop=(kc == 1))
                nc.scalar.activation(keysN[:, c, :], pn[:], Act.Copy)
        f_nodeT()
        f_nodeN()

        # ---- phase 1: LSTM unroll -> HTx ----
        HTx, f_HTx = tc.tile([128, 2, N + 1], fp32, name="HTx" + sx)
        ct, f_ct = tc.tile([128, 2], fp32, name="ct" + sx)
        gsb, f_gsb = tc.tile([128, 8], fp32, name="gsb" + sx)
        sfo, f_sfo = tc.tile([128, 6], fp32, name="sfo" + sx)
        gt, f_gt = tc.tile([128, 2], fp32, name="gt" + sx)
        t1, f_t1 = tc.tile([128, 2], fp32, name="t1" + sx)
        tct, f_tct = tc.tile([128, 2], fp32, name="tct" + sx)
        nc.gpsimd.dma_start(HTx[:, 0, 0:1], h0_d[:, 0:1])
        nc.gpsimd.dma_start(HTx[:, 1, 0:1], h0_d[:, 1:2])
        nc.gpsimd.dma_start(ct[:], c0_d[:, :])
        gpp_cm = tc.tile_pool(name="gpp" + sx, bufs=2, space="PSUM")
        gpp = gpp_cm.__enter__()
        with tc.For_i(0, N, step=U) as iv:
            for u in range(U):
                t = iv + u
                gp = gpp.tile([128, 8], fp32)
                for m2 in range(8):
                    nc.tensor.matmul(gp[:, m2:m2 + 1],
                                     wstat_sb[:, (m2 * 2) * 128:(m2 * 2 + 1) * 128],
                                     HTx[:, 0, ds(t, 1)], start=True, stop=False)
                    nc.tensor.matmul(gp[:, m2:m2 + 1],
                                     wstat_sb[:, (m2 * 2 + 1) * 128:(m2 * 2 + 2) * 128],
                                     HTx[:, 1, ds(t, 1)], start=False, stop=True)
                nc.vector.tensor_tensor(out=gsb[:], in0=gp[:], in1=xb_sb[:], op=Alu.add)
                nc.scalar.activation(sfo[:], gsb[:, 0:6], Act.Sigmoid)
                nc.scalar.activation(gt[:], gsb[:, 6:8], Act.Tanh)
                nc.vector.tensor_tensor(out=t1[:], in0=sfo[:, 0:2], in1=gt[:], op=Alu.mult)
                nc.vector.tensor_tensor(out=ct[:], in0=sfo[:, 2:4], in1=ct[:], op=Alu.mult)
                nc.vector.tensor_tensor(out=ct[:], in0=ct[:], in1=t1[:], op=Alu.add)
                nc.scalar.activation(tct[:], ct[:], Act.Tanh)
                nc.vector.tensor_tensor(out=HTx[:, 0, ds(t + 1, 1)],
                                        in0=sfo[:, 4:5], in1=tct[:, 0:1], op=Alu.mult)
                nc.vector.tensor_tensor(out=HTx[:, 1, ds(t + 1, 1)],
                                        in0=sfo[:, 5:6], in1=tct[:, 1:2], op=Alu.mult)
        gpp_cm.__exit__(None, None, None)

        # ---- QT = Wq @ h + bq (feature-on-partition) ----
        for j2 in range(2):
            for tb in range(4):
                pq = p512.tile([128, 512], fp32, tag="ps")
                for k in range(2):
                    nc.tensor.matmul(pq[:], wqt_sb[:, (k * 2 + j2) * 128:(k * 2 + j2 + 1) * 128],
                                     HTx[:, k, 1 + tb * 512:1 + (tb + 1) * 512],
                                     start=(k == 0), stop=(k == 1))
                nc.vector.tensor_scalar(out=QT[:, j2, tb * 512:(tb + 1) * 512],
                                        in0=pq[:], scalar1=bqt_sb[:, j2:j2 + 1],
                                        scalar2=None, op0=Alu.add)
        f_tct(); f_t1(); f_gt(); f_sfo(); f_gsb(); f_ct(); f_HTx()

        # ---- phase 2+3 interleaved: ST blocks + argmax-rank chain ----
        if OUT_MODE != "tri":
            rb, f_maskb = tc.tile([128, N], fp32, name="rb" + sx)
            trs, f_trs = tc.tile([16, 128], fp32, name="trs" + sx)
        rankn, f_rankn = tc.tile([128, 16], fp32, name="rankn" + sx)
        if OUT_MODE != "tri":
            rr, f_rr = tc.tile([1, N], fp32, name="rr" + sx)
        stp_cm = tc.tile_pool(name="stp" + sx, bufs=2)
        stp = stp_cm.__enter__()
        ma, f_ma = tc.tile([128, 16], fp32, name="ma" + sx)
        ms, f_ms = tc.tile([128, 16], fp32, name="ms" + sx)
        mk, f_mk = tc.tile([128, 16], fp32, name="mk" + sx)
        pm, f_pm = tc.tile([128, 1], fp32, name="pm" + sx)
        gm, f_gm = tc.tile([1, 1], fp32, name="gm" + sx)
        dl, f_dl = tc.tile([128, 16], fp32, name="dl" + sx)
        tpp_cm = tc.tile_pool(name="tpp" + sx, bufs=2, space="PSUM")
        tpp = tpp_cm.__enter__()
        gbp_cm = tc.tile_pool(name="gbp" + sx, bufs=2, space="PSUM")
        gbp = gbp_cm.__enter__()
        nc.vector.memset(ma[:], 0.0)
        nc.vector.memset(ms[:], 0.0)

        def emit_st_block(tb):
            st_tb = stp.tile([128, 16, 512], fp32, name=f"st{tb}" + sx, tag="st")
            for c in range(16):
                pS = p512.tile([128, 512], fp32, tag="ps")
                for k in range(2):
                    nc.tensor.matmul(pS[:], keysT[:, k, c * 128:(c + 1) * 128],
                                     QT[:, k, tb * 512:(tb + 1) * 512],
                                     start=(k == 0), stop=(k == 1))
                nc.scalar.activation(st_tb[:, c, :], pS[:], Act.Copy)
            return st_tb

        def emit_l3(st_tb):
            with tc.For_i(0, 512, step=U) as iv:
                for u in range(U):
                    tl_ = iv + u
                    nc.vector.tensor_tensor(out=mk[:], in0=st_tb[:, :, ds(tl_, 1)],
                                            in1=ma[:], op=Alu.add)
                    # ms += 1 for already-selected boxes (exact small ints)
                    nc.vector.scalar_tensor_tensor(out=ms[:], in0=ma[:],
                                                   scalar=-(2.0 ** -30),
                                                   in1=ms[:], op0=Alu.mult,
                                                   op1=Alu.add)
                    nc.vector.reduce_max(out=pm[:], in_=mk[:], axis=AX.X)
                    tp = tpp.tile([1, 128], fp32, tag="tp")
                    nc.tensor.transpose(tp[:], pm[:], id_sb[:])
                    nc.vector.reduce_max(out=gm[:], in_=tp[:], axis=AX.X)
                    gb = gbp.tile([128, 1], fp32)
                    nc.tensor.matmul(gb[:], ones_sb[:], gm[:], start=True, stop=True)
                    nc.vector.tensor_scalar(out=dl[:], in0=mk[:], scalar1=gb[:],
                                            scalar2=-(2.0 ** 30), op0=Alu.is_ge,
                                            op1=Alu.mult)
                    nc.vector.tensor_tensor(out=ma[:], in0=ma[:], in1=dl[:], op=Alu.add)

        blocks = [emit_st_block(0), emit_st_block(1)]
        emit_l3(blocks[0])
        blocks.append(emit_st_block(2))
        emit_l3(blocks[1])
        blocks.append(emit_st_block(3))
        emit_l3(blocks[2])
        emit_l3(blocks[3])

        # rank_n = 2047 - ms_n (exact small integers)
        nc.vector.tensor_scalar(out=rankn[:], in0=ms[:], scalar1=-1.0,
                                scalar2=2047.0, op0=Alu.mult, op1=Alu.add)

        if OUT_MODE == "tri":
            nc.gpsimd.dma_start(outr_d[:, :], rankn[:])
            # iotab[p, r] = r (broadcast of the column-index row)
            nc.gpsimd.dma_start(iota_sb[:], iota1_d[:, :])
            for g in range(4):
                pr = p512.tile([128, 512], fp32, tag="ps")
                nc.tensor.matmul(pr[:], ones_sb[:], iota_sb[0:1, g * 512:(g + 1) * 512],
                                 start=True, stop=True)
                nc.scalar.activation(iotab[:, g * 512:(g + 1) * 512], pr[:], Act.Copy)
            # K_permT[m, r] = keys[pi(r), m] + bk[m] via one-hot gather matmul:
            # Pi[n, r] = 1{rank[n] == r}; one-hot fp32 matmul copies values exactly.
            pip_cm = tc.tile_pool(name="pip" + sx, bufs=3)
            pip = pip_cm.__enter__()
            for j2 in range(2):
                for rg in range(4):
                    pq = p512.tile([128, 512], fp32, tag="ps")
                    for c in range(16):
                        pi = pip.tile([128, 512], fp32, tag="pi")
                        nc.vector.tensor_scalar(out=pi[:],
                                                in0=iotab[:, rg * 512:(rg + 1) * 512],
                                                scalar1=rankn[:, c:c + 1],
                                                scalar2=None, op0=Alu.is_equal)
                        nc.tensor.matmul(pq[:], keysN[:, c, j2 * 128:(j2 + 1) * 128],
                                         pi[:], start=(c == 0), stop=(c == 15))
                    nc.vector.tensor_scalar(out=KpT[:, j2, rg * 512:(rg + 1) * 512],
                                            in0=pq[:], scalar1=bkt_sb[:, j2:j2 + 1],
                                            scalar2=None, op0=Alu.add)
            pip_cm.__exit__(None, None, None)
            keysP4 = KpT
        else:
            # broadcast rank over rows: rb[p, n] = rank[n]
            tp2 = tpp.tile([16, 128], fp32, tag="tp")
            nc.tensor.transpose(tp2[:], rankn[:], id_sb[:])
            nc.scalar.activation(trs[:], tp2[:], Act.Copy)
            for c in range(16):
                nc.gpsimd.dma_start(rr[0:1, c * 128:(c + 1) * 128], trs[c:c + 1, :])
            for g in range(4):
                pr = p512.tile([128, 512], fp32, tag="ps")
                nc.tensor.matmul(pr[:], ones_sb[:], rr[0:1, g * 512:(g + 1) * 512],
                                 start=True, stop=True)
                nc.scalar.activation(rb[:, g * 512:(g + 1) * 512], pr[:], Act.Copy)
            iotab = rb
            keysP4 = keysT

        gbp_cm.__exit__(None, None, None)
        tpp_cm.__exit__(None, None, None)
        f_dl(); f_gm(); f_pm(); f_mk(); f_ms(); f_ma()
        stp_cm.__exit__(None, None, None)
        if OUT_MODE == "tri":
            f_rankn()
            f_keysT()
            f_keysN()
            f_iota()

        # ---- phase 4: probs rows, masked softmax, DMA out ----
        # mask = (iotab >= t): original column order compares rank[n] >= t;
        # permuted order compares column index r >= t (static triangle).
        rs4, f_rs4 = tc.tile([128, 4], fp32, name="rs4" + sx)
        rsum, f_rsum = tc.tile([128, 1], fp32, name="rsum" + sx)
        rinv, f_rinv = tc.tile([128, 1], fp32, name="rinv" + sx)
        rsc, f_rsc = tc.tile([128, 1], fp32, name="rsc" + sx)
        esp_cm = tc.tile_pool(name="esp" + sx, bufs=2)
        esp = esp_cm.__enter__()
        eop_cm = tc.tile_pool(name="eop" + sx, bufs=2)
        eop = eop_cm.__enter__()
        for blk in range(16):
            es = esp.tile([128, N], fp32, name=f"es{blk}" + sx, tag="es")
            for nb in range(4):
                pS = p512.tile([128, 512], fp32, tag="ps")
                for k in range(2):
                    nc.tensor.matmul(pS[:], QT[:, k, blk * 128:(blk + 1) * 128],
                                     keysP4[:, k, nb * 512:(nb + 1) * 512],
                                     start=(k == 0), stop=(k == 1))
                nc.scalar.activation(es[:, nb * 512:(nb + 1) * 512], pS[:], Act.Exp)
                nc.vector.scalar_tensor_tensor(
                    out=es[:, nb * 512:(nb + 1) * 512],
                    in0=iotab[:, nb * 512:(nb + 1) * 512],
                    scalar=tvs_sb[:, blk:blk + 1],
                    in1=es[:, nb * 512:(nb + 1) * 512],
                    op0=Alu.is_ge, op1=Alu.mult,
                    accum_out=rs4[:, nb:nb + 1])
            nc.vector.reduce_sum(out=rsum[:], in_=rs4[:], axis=AX.X)
            nc.vector.reciprocal(rinv[:], rsum[:])
            eo = eop.tile([128, N], odt, name=f"eo{blk}" + sx, tag="eo")
            if OUT_MODE in ("tri", "u8"):
                nc.vector.tensor_scalar(out=rsc[:], in0=rinv[:], scalar1=OUT_SCALE,
                                        scalar2=None, op0=Alu.mult)
                for nb in range(4):
                    nc.scalar.activation(eo[:, nb * 512:(nb + 1) * 512],
                                         es[:, nb * 512:(nb + 1) * 512],
                                         Act.Copy, scale=rsc[:])
            else:
                for nb in range(4):
                    nc.scalar.activation(eo[:, nb * 512:(nb + 1) * 512],
                                         es[:, nb * 512:(nb + 1) * 512],
                                         Act.Copy, scale=rinv[:])
            if OUT_MODE == "tri":
                for sub in range(128 // RB):
                    i = blk * (128 // RB) + sub  # global band index
                    nc.gpsimd.dma_start(outp_d[:, CO[i]:CO[i] + BW[i]],
                                        eo[sub * RB:(sub + 1) * RB, i * RB:N])
            else:
                nc.gpsimd.dma_start(out_d[blk * 128:(blk + 1) * 128, :], eo[:])
        eop_cm.__exit__(None, None, None)
        esp_cm.__exit__(None, None, None)
        f_rsc(); f_rinv(); f_rsum(); f_rs4()
        if OUT_MODE == "tri":
            f_iotab(); f_KpT(); f_QT()
        else:
            f_rr(); f_rankn(); f_trs(); f_maskb()
            f_keysT(); f_QT()
        p512_cm.__exit__(None, None, None)
        f_tvs(); f_id(); f_ones(); f_bqt(); f_bkt(); f_wqt(); f_wkt(); f_wstat(); f_xb()

    from concourse import tile
    with tile.TileContext(nc) as tc:
        emit_pass(tc, 0)

    nc.compile()
    _cache["nc"] = nc
    return nc


def _prep_weights(inputs):
    """Per-core-identical operands, keyed by dram tensor name (per-core shapes)."""
    f32 = np.float32
    decoder_init = np.asarray(inputs["decoder_init"], dtype=f32)
    hidden0 = np.asarray(inputs["hidden0"], dtype=f32)
    w_ih = np.asarray(inputs["w_ih"], dtype=f32)
    w_hh = np.asarray(inputs["w_hh"], dtype=f32)
    b_ih = np.asarray(inputs["b_ih"], dtype=f32)
    b_hh = np.asarray(inputs["b_hh"], dtype=f32)
    Wq = np.asarray(inputs["Wq"], dtype=f32)
    bq = np.asarray(inputs["bq"], dtype=f32)
    Wk = np.asarray(inputs["Wk"], dtype=f32)
    bk = np.asarray(inputs["bk"], dtype=f32)

    perm = np.concatenate([np.arange(0, 256), np.arange(256, 512),
                           np.arange(768, 1024), np.arange(512, 768)])
    w_hh_p = w_hh[perm]
    x_proj = decoder_init @ w_ih.T + b_ih
    xb = np.ascontiguousarray(((x_proj + b_hh)[perm]).reshape(8, 128).T, dtype=f32)
    wstat = np.zeros((128, 2048), f32)
    for m2 in range(8):
        for k in range(2):
            blockT = w_hh_p[m2 * 128:(m2 + 1) * 128, k * 128:(k + 1) * 128].T
            wstat[:, (m2 * 2 + k) * 128:(m2 * 2 + k + 1) * 128] = blockT
    WkT = Wk.T
    WqT = Wq.T
    wkt = np.zeros((128, 512), f32)
    wqt = np.zeros((128, 512), f32)
    for k in range(2):
        for j in range(2):
            wkt[:, (k * 2 + j) * 128:(k * 2 + j + 1) * 128] = \
                WkT[k * 128:(k + 1) * 128, j * 128:(j + 1) * 128]
            wqt[:, (k * 2 + j) * 128:(k * 2 + j + 1) * 128] = \
                WqT[k * 128:(k + 1) * 128, j * 128:(j + 1) * 128]
    bkt = np.ascontiguousarray(bk.reshape(2, 128).T, dtype=f32)
    bqt = np.ascontiguousarray(bq.reshape(2, 128).T, dtype=f32)
    h0c = np.ascontiguousarray(hidden0.reshape(2, 128).T, dtype=f32)
    ones1 = np.ones((1, 128), f32)
    ident = np.eye(128, dtype=f32)
    tvs = (np.arange(128, dtype=f32)[:, None] +
           128.0 * np.arange(16, dtype=f32)[None, :]).astype(f32)
    w = dict(xb=xb, wstat=wstat, wkt=wkt, wqt=wqt, bkt=bkt, bqt=bqt,
             ones1=ones1, ident=ident, tvs=tvs, h0=h0c)
    if OUT_MODE == "tri":
        w["iota1"] = np.arange(N, dtype=f32)[None, :]
    return w


def _weights_key(inputs):
    h = hashlib.blake2b(digest_size=16)
    for k in ("decoder_init", "hidden0", "w_ih", "w_hh", "b_ih", "b_hh",
              "Wq", "bq", "Wk", "bk"):
        a = np.ascontiguousarray(np.asarray(inputs[k], dtype=np.float32))
        h.update(a.tobytes())
    return h.digest()


def _get_rt():
    if "rt" in _cache:
        return _cache["rt"]
    nc = _build()
    import concourse.mybir as mybir
    from concourse.bass2jax import (_bass_exec_p, install_neuronx_cc_hook,
                                    partition_id_tensor)
    import jax
    import jax.numpy as jnp
    from jax.sharding import Mesh, PartitionSpec, NamedSharding
    from jax.experimental.shard_map import shard_map

    install_neuronx_cc_hook()
    partition_name = nc.partition_id_tensor.name if nc.partition_id_tensor else None

    in_names = []
    out_names = []
    out_avals = []
    for alloc in nc.m.functions[0].allocations:
        if not isinstance(alloc, mybir.MemoryLocationSet):
            continue
        name = alloc.memorylocations[0].name
        if alloc.kind == "ExternalInput":
            if name != partition_name:
                in_names.append(name)
        elif alloc.kind == "ExternalOutput":
            out_names.append(name)
            out_avals.append(jax.core.ShapedArray(tuple(alloc.tensor_shape),
                                                  mybir.dt.np(alloc.dtype)))
    n_params = len(in_names)
    n_outs = len(out_avals)
    in_names_full = list(in_names) + out_names
    if partition_name is not None:
        in_names_full.append(partition_name)
    donate = tuple(range(n_params, n_params + n_outs))

    def _body(*args):
        operands = list(args)
        if partition_name is not None:
            operands.append(partition_id_tensor())
        return tuple(_bass_exec_p.bind(
            *operands, out_avals=tuple(out_avals), in_names=tuple(in_names_full),
            out_names=tuple(out_names), lowering_input_output_aliases=(),
            sim_require_finite=True, sim_require_nnan=True, nc=nc))

    devices = jax.devices()[:NCORES]
    gsz = NCORES // GROUPS
    groups = []
    in_specs = (PartitionSpec("core"),) * (n_params + n_outs)
    out_specs = (PartitionSpec("core"),) * n_outs
    for g in range(GROUPS):
        devs = devices[g * gsz:(g + 1) * gsz]
        mesh = Mesh(np.asarray(devs), ("core",))
        sh = NamedSharding(mesh, PartitionSpec("core"))
        sharded = jax.jit(
            shard_map(_body, mesh=mesh, in_specs=in_specs, out_specs=out_specs,
                      check_rep=False),
            donate_argnums=donate, keep_unused=True)
        zeros_jit = jax.jit(
            lambda gsz=gsz: tuple(
                jnp.zeros((gsz * av.shape[0], *av.shape[1:]), av.dtype)
                for av in out_avals),
            out_shardings=(sh,) * n_outs)
        groups.append(SimpleNamespace(devs=devs, sh=sh, sharded=sharded,
                                      zeros_jit=zeros_jit))

    rt = SimpleNamespace(nc=nc, jax=jax, in_names=in_names, out_names=out_names,
                         groups=groups, gsz=gsz,
                         weights_dev=None, weights_key=None,
                         pool=ThreadPoolExecutor(max_workers=32))
    _cache["rt"] = rt
    return rt


def _upload_weights(rt, inputs):
    key = _weights_key(inputs)
    if rt.weights_key == key and rt.weights_dev is not None:
        return rt.weights_dev
    w = _prep_weights(inputs)
    dev = []
    for grp in rt.groups:
        gd = {}
        for name, arr in w.items():
            t = np.tile(arr, (rt.gsz, 1))
            gd[name] = rt.jax.device_put(t, grp.sh)
        dev.append(gd)
    for gd in dev:
        rt.jax.block_until_ready(list(gd.values()))
    rt.weights_dev = dev
    rt.weights_key = key
    return dev


def _run(inputs, trace=False, tmpdir=None):
    rt = _get_rt()
    jax = rt.jax
    f32 = np.float32
    gsz = rt.gsz

    weights = _upload_weights(rt, inputs)

    node_embedding = np.asarray(inputs["node_embedding"], dtype=f32)
    z_g = np.asarray(inputs["z_g"], dtype=f32)
    c0_all = np.ascontiguousarray(
        z_g.reshape(NCORES, 2, 128).transpose(0, 2, 1)).reshape(NCORES * 128, 2)

    H = N // 2
    res = np.empty((B, N, N), f32)
    inv_scale = f32(1.0 / OUT_SCALE)
    recon_futs = []
    prev_nodes = None

    for g, grp in enumerate(rt.groups):
        if prev_nodes is not None:
            # stagger: keep the half-duplex link dedicated to the previous
            # group's upload; its exec+download then overlaps our upload
            jax.block_until_ready(prev_nodes)
        cores = list(range(g * gsz, (g + 1) * gsz))
        # per-call activations upload, shard-parallel (node split into two
        # tensors so 2*gsz transfers run concurrently)
        futs = {(b, j): rt.pool.submit(
                    jax.device_put, node_embedding[b, j * H:(j + 1) * H],
                    grp.devs[b - g * gsz])
                for b in cores for j in range(2)}
        zeros = grp.zeros_jit()
        node_arrs = [
            jax.make_array_from_single_device_arrays(
                (gsz * H, M), grp.sh, [futs[(b, j)].result() for b in cores])
            for j in range(2)]
        c0_g = c0_all[g * gsz * 128:(g + 1) * gsz * 128]
        per_call = {"node_a": node_arrs[0], "node_b": node_arrs[1], "c0": c0_g}
        args = [per_call[nm] if nm in per_call else weights[g][nm]
                for nm in rt.in_names]
        outs = grp.sharded(*args, *zeros)
        out_by_name = dict(zip(rt.out_names, outs))

        if OUT_MODE == "tri":
            ranks_fut = rt.pool.submit(np.asarray, out_by_name["outr"])

            def _recon(s, g=g, ranks_fut=ranks_fut):
                bl = s.index[0].start // RB
                buf = np.asarray(s.data)  # [RB, CW] u8
                rankn = ranks_fut.result()[bl * 128:(bl + 1) * 128]  # [128, 16]
                rank = rankn.T.reshape(N).astype(np.int64)  # rank[n]
                P = np.zeros((N, N), np.uint8)
                for i in range(NB):
                    P[i * RB:(i + 1) * RB, i * RB:] = buf[:, CO[i]:CO[i] + BW[i]]
                a = np.take(P, rank, axis=1)
                np.multiply(a, inv_scale, out=res[g * gsz + bl])

            recon_futs += [rt.pool.submit(_recon, s)
                           for s in out_by_name["outp"].addressable_shards]
        else:
            out_arr = out_by_name["out"]

            def _fetch(task, g=g):
                s, j = task
                bl = s.index[0].start // N
                a = np.asarray(s.data[j * H:(j + 1) * H])
                dst = res[g * gsz + bl, j * H:(j + 1) * H]
                if OUT_MODE == "u8":
                    np.multiply(a, inv_scale, out=dst)
                else:
                    np.copyto(dst, a, casting="unsafe")

            recon_futs += [rt.pool.submit(_fetch, (s, j))
                           for s in out_arr.addressable_shards for j in range(2)]
        prev_nodes = node_arrs

    for f in recon_futs:
        f.result()
    return res, SimpleNamespace(exec_time_ns=None, results=None)


def kernel(**inputs) -> np.ndarray:
    out, _ = _run(inputs, trace=False)
    return out


# revision 29
# speedup vs baseline: 1.0211x; 1.0211x over previous
import sys
import hashlib
from types import SimpleNamespace
from concurrent.futures import ThreadPoolExecutor

import numpy as np

sys.path.insert(0, "/opt/trn_rl_repo")

B, N, M = 8, 2048, 256
NCORES = 8
U = 64  # unroll factor inside hardware loops

# Output encoding:
#  "tri": permute probs columns by selection order on-device -> masked zeros
#         become a static lower triangle; ship only the packed upper triangle
#         (u8-quantized) + the rank vector, reconstruct on the host.
#  "u8":  full [N,N] probs matrix quantized to uint8.
#  probs are in [0,1]; code = round(p*OUT_SCALE) on the Act engine (round-to-
#  nearest cast), decode v/OUT_SCALE on host: quant err <= 0.5/253 ~ 2e-3.
OUT_MODE = "tri"  # "tri" | "u8" | "f16" | "f32"
OUT_SCALE = 253.0
# Pipeline groups: cores can be split into GROUPS sets run as separate
# staggered dispatches (group A's exec+download overlapping group B's upload).
# Measured: GROUPS=2 loses ~30ms — the stagger's readiness-sync RPC outweighs
# the hidden exec gap on this tunnel. Keep the single-dispatch path.
GROUPS = 1
# Block-packed triangle: each 32-row band [t0, t0+32) keeps columns [t0, N) —
# the static lower-triangle rest is exactly zero and is dropped. 2.13MB/core
# vs 4.2MB dense, with 64 rectangular DMAs per core.
RB = 32  # band height
NB = N // RB  # 64 bands
BW = [N - i * RB for i in range(NB)]  # band widths
CO = [sum(BW[:i]) for i in range(NB)]  # band column offsets in packed layout
CW = sum(BW)  # packed columns per 32-row band layer

_cache = {}


def _build():
    if "nc" in _cache:
        return _cache["nc"]
    from concourse import bass, tile, bacc
    import concourse.mybir as mybir

    fp32 = mybir.dt.float32
    u8 = mybir.dt.uint8
    odt = {"tri": u8, "u8": u8, "f16": mybir.dt.float16,
           "f32": fp32}[OUT_MODE]
    Alu = mybir.AluOpType
    Act = mybir.ActivationFunctionType
    AX = mybir.AxisListType
    ds = bass.ds

    nc = bacc.Bacc("TRN2", target_bir_lowering=False, debug=False,
                   num_devices=NCORES)

    node_a = nc.dram_tensor("node_a", [N // 2, M], fp32, kind="ExternalInput").ap()
    node_b = nc.dram_tensor("node_b", [N // 2, M], fp32, kind="ExternalInput").ap()
    c0_d = nc.dram_tensor("c0", [128, 2], fp32, kind="ExternalInput").ap()
    h0_d = nc.dram_tensor("h0", [128, 2], fp32, kind="ExternalInput").ap()
    xb_d = nc.dram_tensor("xb", [128, 8], fp32, kind="ExternalInput").ap()
    wstat_d = nc.dram_tensor("wstat", [128, 2048], fp32, kind="ExternalInput").ap()
    wkt_d = nc.dram_tensor("wkt", [128, 512], fp32, kind="ExternalInput").ap()
    wqt_d = nc.dram_tensor("wqt", [128, 512], fp32, kind="ExternalInput").ap()
    bkt_d = nc.dram_tensor("bkt", [128, 2], fp32, kind="ExternalInput").ap()
    bqt_d = nc.dram_tensor("bqt", [128, 2], fp32, kind="ExternalInput").ap()
    ones_d = nc.dram_tensor("ones1", [1, 128], fp32, kind="ExternalInput").ap()
    id_d = nc.dram_tensor("ident", [128, 128], fp32, kind="ExternalInput").ap()
    tvs_d = nc.dram_tensor("tvs", [128, 16], fp32, kind="ExternalInput").ap()
    if OUT_MODE == "tri":
        iota1_d = nc.dram_tensor("iota1", [1, N], fp32, kind="ExternalInput").ap()
        outp_d = nc.dram_tensor("outp", [RB, CW], u8, kind="ExternalOutput").ap()
        outr_d = nc.dram_tensor("outr", [128, 16], fp32, kind="ExternalOutput").ap()
        out_d = None
    else:
        out_d = nc.dram_tensor("out", [N, N], odt, kind="ExternalOutput").ap()

    def emit_pass(tc, rep):
        sx = f"_r{rep}"
        # ---- constants in SBUF ----
        xb_sb, f_xb = tc.tile([128, 8], fp32, name="xb_sb" + sx)
        wstat_sb, f_wstat = tc.tile([128, 2048], fp32, name="wstat_sb" + sx)
        wkt_sb, f_wkt = tc.tile([128, 512], fp32, name="wkt_sb" + sx)
        wqt_sb, f_wqt = tc.tile([128, 512], fp32, name="wqt_sb" + sx)
        bkt_sb, f_bkt = tc.tile([128, 2], fp32, name="bkt_sb" + sx)
        bqt_sb, f_bqt = tc.tile([128, 2], fp32, name="bqt_sb" + sx)
        ones_sb, f_ones = tc.tile([1, 128], fp32, name="ones_sb" + sx)
        id_sb, f_id = tc.tile([128, 128], fp32, name="id_sb" + sx)
        tvs_sb, f_tvs = tc.tile([128, 16], fp32, name="tvs_sb" + sx)
        for sb, dr in ((xb_sb, xb_d), (wstat_sb, wstat_d), (wkt_sb, wkt_d),
                       (wqt_sb, wqt_d), (bkt_sb, bkt_d), (bqt_sb, bqt_d),
                       (ones_sb, ones_d), (id_sb, id_d), (tvs_sb, tvs_d)):
            nc.gpsimd.dma_start(sb[:], dr[:, :])

        # main PSUM pool used across all phases
        p512_cm = tc.tile_pool(name="p512" + sx, bufs=2, space="PSUM")
        p512 = p512_cm.__enter__()

        # ---- persistent big tensors ----
        # Alloc order is load-bearing: the tile allocator releases strictly
        # LIFO, so tiles that die earliest are allocated last.
        QT, f_QT = tc.tile([128, 2, N], fp32, name="QT" + sx)
        if OUT_MODE == "tri":
            KpT, f_KpT = tc.tile([128, 2, N], fp32, name="KpT" + sx)
            iotab, f_iotab = tc.tile([128, N], fp32, name="iotab" + sx)
            iota_sb, f_iota = tc.tile([1, N], fp32, name="iota_sb" + sx)
            keysN, f_keysN = tc.tile([128, 16, 256], fp32, name="keysN" + sx)
        keysT, f_keysT = tc.tile([128, 2, N], fp32, name="keysT" + sx)

        # ---- prologue: node -> nodeT -> keysT (+ keysN for tri) ----
        nodeN, f_nodeN = tc.tile([128, 16, 256], fp32, name="nodeN" + sx)
        nodeT, f_nodeT = tc.tile([128, 2, N], fp32, name="nodeT" + sx)
        for c in range(16):
            src = node_a if c < 8 else node_b
            c0_ = c if c < 8 else c - 8
            nc.gpsimd.dma_start(nodeN[:, c, :], src[c0_ * 128:(c0_ + 1) * 128, :])
        ptp_cm = tc.tile_pool(name="ptp" + sx, bufs=2, space="PSUM")
        ptp = ptp_cm.__enter__()
        for c in range(16):
            for k in range(2):
                pt = ptp.tile([128, 128], fp32)
                nc.tensor.transpose(pt[:], nodeN[:, c, k * 128:(k + 1) * 128], id_sb[:])
                nc.scalar.activation(nodeT[:, k, c * 128:(c + 1) * 128], pt[:], Act.Copy)
        ptp_cm.__exit__(None, None, None)
        for j2 in range(2):
            for nb in range(4):
                pk = p512.tile([128, 512], fp32, tag="ps")
                for k in range(2):
                    nc.tensor.matmul(pk[:], wkt_sb[:, (k * 2 + j2) * 128:(k * 2 + j2 + 1) * 128],
                                     nodeT[:, k, nb * 512:(nb + 1) * 512],
                                     start=(k == 0), stop=(k == 1))
                nc.vector.tensor_scalar(out=keysT[:, j2, nb * 512:(nb + 1) * 512],
                                        in0=pk[:], scalar1=bkt_sb[:, j2:j2 + 1],
                                        scalar2=None, op0=Alu.add)
        if OUT_MODE == "tri":
            # keysN[n(p), m(f)] = (node @ Wk^T), unbiased, n-on-partition.
            # Used later to gather keys by selection order via a one-hot matmul.
            for c in range(16):
                pn = p512.tile([128, 256], fp32, tag="ps")
                for kc in range(2):
                    nc.tensor.matmul(pn[:], nodeT[:, kc, c * 128:(c + 1) * 128],
                                     wkt_sb[:, kc * 256:(kc + 1) * 256],
                                     start=(kc == 0), stop=(kc == 1))
                nc.scalar.activation(keysN[:, c, :], pn[:], Act.Copy)
        f_nodeT()
        f_nodeN()

        # ---- phase 1: LSTM unroll -> HTx ----
        HTx, f_HTx = tc.tile([128, 2, N + 1], fp32, name="HTx" + sx)
        ct, f_ct = tc.tile([128, 2], fp32, name="ct" + sx)
        gsb, f_gsb = tc.tile([128, 8], fp32, name="gsb" + sx)
        sfo, f_sfo = tc.tile([128, 6], fp32, name="sfo" + sx)
        gt, f_gt = tc.tile([128, 2], fp32, name="gt" + sx)
        t1, f_t1 = tc.tile([128, 2], fp32, name="t1" + sx)
        tct, f_tct = tc.tile([128, 2], fp32, name="tct" + sx)
        nc.gpsimd.dma_start(HTx[:, 0, 0:1], h0_d[:, 0:1])
        nc.gpsimd.dma_start(HTx[:, 1, 0:1], h0_d[:, 1:2])
        nc.gpsimd.dma_start(ct[:], c0_d[:, :])
        gpp_cm = tc.tile_pool(name="gpp" + sx, bufs=2, space="PSUM")
        gpp = gpp_cm.__enter__()
        with tc.For_i(0, N, step=U) as iv:
            for u in range(U):
                t = iv + u
                gp = gpp.tile([128, 8], fp32)
                for m2 in range(8):
                    nc.tensor.matmul(gp[:, m2:m2 + 1],
                                     wstat_sb[:, (m2 * 2) * 128:(m2 * 2 + 1) * 128],
                                     HTx[:, 0, ds(t, 1)], start=True, stop=False)
                    nc.tensor.matmul(gp[:, m2:m2 + 1],
                                     wstat_sb[:, (m2 * 2 + 1) * 128:(m2 * 2 + 2) * 128],
                                     HTx[:, 1, ds(t, 1)], start=False, stop=True)
                nc.vector.tensor_tensor(out=gsb[:], in0=gp[:], in1=xb_sb[:], op=Alu.add)
                nc.scalar.activation(sfo[:], gsb[:, 0:6], Act.Sigmoid)
                nc.scalar.activation(gt[:], gsb[:, 6:8], Act.Tanh)
                nc.vector.tensor_tensor(out=t1[:], in0=sfo[:, 0:2], in1=gt[:], op=Alu.mult)
                nc.vector.tensor_tensor(out=ct[:], in0=sfo[:, 2:4], in1=ct[:], op=Alu.mult)
                nc.vector.tensor_tensor(out=ct[:], in0=ct[:], in1=t1[:], op=Alu.add)
                nc.scalar.activation(tct[:], ct[:], Act.Tanh)
                nc.vector.tensor_tensor(out=HTx[:, 0, ds(t + 1, 1)],
                                        in0=sfo[:, 4:5], in1=tct[:, 0:1], op=Alu.mult)
                nc.vector.tensor_tensor(out=HTx[:, 1, ds(t + 1, 1)],
                                        in0=sfo[:, 5:6], in1=tct[:, 1:2], op=Alu.mult)
        gpp_cm.__exit__(None, None, None)

        # ---- QT = Wq @ h + bq (feature-on-partition) ----
        for j2 in range(2):
            for tb in range(4):
                pq = p512.tile([128, 512], fp32, tag="ps")
                for k in range(2):
                    nc.tensor.matmul(pq[:], wqt_sb[:, (k * 2 + j2) * 128:(k * 2 + j2 + 1) * 128],
                                     HTx[:, k, 1 + tb * 512:1 + (tb + 1) * 512],
                                     start=(k == 0), stop=(k == 1))
                nc.vector.tensor_scalar(out=QT[:, j2, tb * 512:(tb + 1) * 512],
                                        in0=pq[:], scalar1=bqt_sb[:, j2:j2 + 1],
                                        scalar2=None, op0=Alu.add)
        f_tct(); f_t1(); f_gt(); f_sfo(); f_gsb(); f_ct(); f_HTx()

        # ---- phase 2+3 interleaved: ST blocks + argmax-rank chain ----
        if OUT_MODE != "tri":
            rb, f_maskb = tc.tile([128, N], fp32, name="rb" + sx)
            trs, f_trs = tc.tile([16, 128], fp32, name="trs" + sx)
        rankn, f_rankn = tc.tile([128, 16], fp32, name="rankn" + sx)
        if OUT_MODE != "tri":
            rr, f_rr = tc.tile([1, N], fp32, name="rr" + sx)
        stp_cm = tc.tile_pool(name="stp" + sx, bufs=2)
        stp = stp_cm.__enter__()
        ma, f_ma = tc.tile([128, 16], fp32, name="ma" + sx)
        ms, f_ms = tc.tile([128, 16], fp32, name="ms" + sx)
        mk, f_mk = tc.tile([128, 16], fp32, name="mk" + sx)
        pm, f_pm = tc.tile([128, 1], fp32, name="pm" + sx)
        gm, f_gm = tc.tile([1, 1], fp32, name="gm" + sx)
        dl, f_dl = tc.tile([128, 16], fp32, name="dl" + sx)
        tpp_cm = tc.tile_pool(name="tpp" + sx, bufs=2, space="PSUM")
        tpp = tpp_cm.__enter__()
        gbp_cm = tc.tile_pool(name="gbp" + sx, bufs=2, space="PSUM")
        gbp = gbp_cm.__enter__()
        nc.vector.memset(ma[:], 0.0)
        nc.vector.memset(ms[:], 0.0)

        def emit_st_block(tb):
            st_tb = stp.tile([128, 16, 512], fp32, name=f"st{tb}" + sx, tag="st")
            for c in range(16):
                pS = p512.tile([128, 512], fp32, tag="ps")
                for k in range(2):
                    nc.tensor.matmul(pS[:], keysT[:, k, c * 128:(c + 1) * 128],
                                     QT[:, k, tb * 512:(tb + 1) * 512],
                                     start=(k == 0), stop=(k == 1))
                nc.scalar.activation(st_tb[:, c, :], pS[:], Act.Copy)
            return st_tb

        def emit_l3(st_tb):
            with tc.For_i(0, 512, step=U) as iv:
                for u in range(U):
                    tl_ = iv + u
                    nc.vector.tensor_tensor(out=mk[:], in0=st_tb[:, :, ds(tl_, 1)],
                                            in1=ma[:], op=Alu.add)
                    # ms += 1 for already-selected boxes (exact small ints)
                    nc.vector.scalar_tensor_tensor(out=ms[:], in0=ma[:],
                                                   scalar=-(2.0 ** -30),
                                                   in1=ms[:], op0=Alu.mult,
                                                   op1=Alu.add)
                    nc.vector.reduce_max(out=pm[:], in_=mk[:], axis=AX.X)
                    tp = tpp.tile([1, 128], fp32, tag="tp")
                    nc.tensor.transpose(tp[:], pm[:], id_sb[:])
                    nc.vector.reduce_max(out=gm[:], in_=tp[:], axis=AX.X)
                    gb = gbp.tile([128, 1], fp32)
                    nc.tensor.matmul(gb[:], ones_sb[:], gm[:], start=True, stop=True)
                    nc.vector.tensor_scalar(out=dl[:], in0=mk[:], scalar1=gb[:],
                                            scalar2=-(2.0 ** 30), op0=Alu.is_ge,
                                            op1=Alu.mult)
                    nc.vector.tensor_tensor(out=ma[:], in0=ma[:], in1=dl[:], op=Alu.add)

        blocks = [emit_st_block(0), emit_st_block(1)]
        emit_l3(blocks[0])
        blocks.append(emit_st_block(2))
        emit_l3(blocks[1])
        blocks.append(emit_st_block(3))
        emit_l3(blocks[2])
        emit_l3(blocks[3])

        # rank_n = 2047 - ms_n (exact small integers)
        nc.vector.tensor_scalar(out=rankn[:], in0=ms[:], scalar1=-1.0,
                                scalar2=2047.0, op0=Alu.mult, op1=Alu.add)

        if OUT_MODE == "tri":
            nc.gpsimd.dma_start(outr_d[:, :], rankn[:])
            # iotab[p, r] = r (broadcast of the column-index row)
            nc.gpsimd.dma_start(iota_sb[:], iota1_d[:, :])
            for g in range(4):
                pr = p512.tile([128, 512], fp32, tag="ps")
                nc.tensor.matmul(pr[:], ones_sb[:], iota_sb[0:1, g * 512:(g + 1) * 512],
                                 start=True, stop=True)
                nc.scalar.activation(iotab[:, g * 512:(g + 1) * 512], pr[:], Act.Copy)
            # K_permT[m, r] = keys[pi(r), m] + bk[m] via one-hot gather matmul:
            # Pi[n, r] = 1{rank[n] == r}; one-hot fp32 matmul copies values exactly.
            pip_cm = tc.tile_pool(name="pip" + sx, bufs=3)
            pip = pip_cm.__enter__()
            for j2 in range(2):
                for rg in range(4):
                    pq = p512.tile([128, 512], fp32, tag="ps")
                    for c in range(16):
                        pi = pip.tile([128, 512], fp32, tag="pi")
                        nc.vector.tensor_scalar(out=pi[:],
                                                in0=iotab[:, rg * 512:(rg + 1) * 512],
                                                scalar1=rankn[:, c:c + 1],
                                                scalar2=None, op0=Alu.is_equal)
                        nc.tensor.matmul(pq[:], keysN[:, c, j2 * 128:(j2 + 1) * 128],
                                         pi[:], start=(c == 0), stop=(c == 15))
                    nc.vector.tensor_scalar(out=KpT[:, j2, rg * 512:(rg + 1) * 512],
                                            in0=pq[:], scalar1=bkt_sb[:, j2:j2 + 1],
                                            scalar2=None, op0=Alu.add)
            pip_cm.__exit__(None, None, None)
            keysP4 = KpT
        else:
            # broadcast rank over rows: rb[p, n] = rank[n]
            tp2 = tpp.tile([16, 128], fp32, tag="tp")
            nc.tensor.transpose(tp2[:], rankn[:], id_sb[:])
            nc.scalar.activation(trs[:], tp2[:], Act.Copy)
            for c in range(16):
                nc.gpsimd.dma_start(rr[0:1, c * 128:(c + 1) * 128], trs[c:c + 1, :])
            for g in range(4):
                pr = p512.tile([128, 512], fp32, tag="ps")
                nc.tensor.matmul(pr[:], ones_sb[:], rr[0:1, g * 512:(g + 1) * 512],
                                 start=True, stop=True)
                nc.scalar.activation(rb[:, g * 512:(g + 1) * 512], pr[:], Act.Copy)
            iotab = rb
            keysP4 = keysT

        gbp_cm.__exit__(None, None, None)
        tpp_cm.__exit__(None, None, None)
        f_dl(); f_gm(); f_pm(); f_mk(); f_ms(); f_ma()
        stp_cm.__exit__(None, None, None)
        if OUT_MODE == "tri":
            f_rankn()
            f_keysT()
            f_keysN()
            f_iota()

        # ---- phase 4: probs rows, masked softmax, DMA out ----
        # mask = (iotab >= t): original column order compares rank[n] >= t;
        # permuted order compares column index r >= t (static triangle).
        rs4, f_rs4 = tc.tile([128, 4], fp32, name="rs4" + sx)
        rsum, f_rsum = tc.tile([128, 1], fp32, name="rsum" + sx)
        rinv, f_rinv = tc.tile([128, 1], fp32, name="rinv" + sx)
        rsc, f_rsc = tc.tile([128, 1], fp32, name="rsc" + sx)
        esp_cm = tc.tile_pool(name="esp" + sx, bufs=2)
        esp = esp_cm.__enter__()
        eop_cm = tc.tile_pool(name="eop" + sx, bufs=2)
        eop = eop_cm.__enter__()
        for blk in range(16):
            es = esp.tile([128, N], fp32, name=f"es{blk}" + sx, tag="es")
            for nb in range(4):
                pS = p512.tile([128, 512], fp32, tag="ps")
                for k in range(2):
                    nc.tensor.matmul(pS[:], QT[:, k, blk * 128:(blk + 1) * 128],
                                     keysP4[:, k, nb * 512:(nb + 1) * 512],
                                     start=(k == 0), stop=(k == 1))
                nc.scalar.activation(es[:, nb * 512:(nb + 1) * 512], pS[:], Act.Exp)
                nc.vector.scalar_tensor_tensor(
                    out=es[:, nb * 512:(nb + 1) * 512],
                    in0=iotab[:, nb * 512:(nb + 1) * 512],
                    scalar=tvs_sb[:, blk:blk + 1],
                    in1=es[:, nb * 512:(nb + 1) * 512],
                    op0=Alu.is_ge, op1=Alu.mult,
                    accum_out=rs4[:, nb:nb + 1])
            nc.vector.reduce_sum(out=rsum[:], in_=rs4[:], axis=AX.X)
            nc.vector.reciprocal(rinv[:], rsum[:])
            eo = eop.tile([128, N], odt, name=f"eo{blk}" + sx, tag="eo")
            if OUT_MODE in ("tri", "u8"):
                nc.vector.tensor_scalar(out=rsc[:], in0=rinv[:], scalar1=OUT_SCALE,
                                        scalar2=None, op0=Alu.mult)
                for nb in range(4):
                    nc.scalar.activation(eo[:, nb * 512:(nb + 1) * 512],
                                         es[:, nb * 512:(nb + 1) * 512],
                                         Act.Copy, scale=rsc[:])
            else:
                for nb in range(4):
                    nc.scalar.activation(eo[:, nb * 512:(nb + 1) * 512],
                                         es[:, nb * 512:(nb + 1) * 512],
                                         Act.Copy, scale=rinv[:])
            if OUT_MODE == "tri":
                for sub in range(128 // RB):
                    i = blk * (128 // RB) + sub  # global band index
                    nc.gpsimd.dma_start(outp_d[:, CO[i]:CO[i] + BW[i]],
                                        eo[sub * RB:(sub + 1) * RB, i * RB:N])
            else:
                nc.gpsimd.dma_start(out_d[blk * 128:(blk + 1) * 128, :], eo[:])
        eop_cm.__exit__(None, None, None)
        esp_cm.__exit__(None, None, None)
        f_rsc(); f_rinv(); f_rsum(); f_rs4()
        if OUT_MODE == "tri":
            f_iotab(); f_KpT(); f_QT()
        else:
            f_rr(); f_rankn(); f_trs(); f_maskb()
            f_keysT(); f_QT()
        p512_cm.__exit__(None, None, None)
        f_tvs(); f_id(); f_ones(); f_bqt(); f_bkt(); f_wqt(); f_wkt(); f_wstat(); f_xb()

    from concourse import tile
    with tile.TileContext(nc) as tc:
        emit_pass(tc, 0)

    nc.compile()
    _cache["nc"] = nc
    return nc


def _prep_weights(inputs):
    """Per-core-identical operands, keyed by dram tensor name (per-core shapes)."""
    f32 = np.float32
    decoder_init = np.asarray(inputs["decoder_init"], dtype=f32)
    hidden0 = np.asarray(inputs["hidden0"], dtype=f32)
    w_ih = np.asarray(inputs["w_ih"], dtype=f32)
    w_hh = np.asarray(inputs["w_hh"], dtype=f32)
    b_ih = np.asarray(inputs["b_ih"], dtype=f32)
    b_hh = np.asarray(inputs["b_hh"], dtype=f32)
    Wq = np.asarray(inputs["Wq"], dtype=f32)
    bq = np.asarray(inputs["bq"], dtype=f32)
    Wk = np.asarray(inputs["Wk"], dtype=f32)
    bk = np.asarray(inputs["bk"], dtype=f32)

    perm = np.concatenate([np.arange(0, 256), np.arange(256, 512),
                           np.arange(768, 1024), np.arange(512, 768)])
    w_hh_p = w_hh[perm]
    x_proj = decoder_init @ w_ih.T + b_ih
    xb = np.ascontiguousarray(((x_proj + b_hh)[perm]).reshape(8, 128).T, dtype=f32)
    wstat = np.zeros((128, 2048), f32)
    for m2 in range(8):
        for k in range(2):
            blockT = w_hh_p[m2 * 128:(m2 + 1) * 128, k * 128:(k + 1) * 128].T
            wstat[:, (m2 * 2 + k) * 128:(m2 * 2 + k + 1) * 128] = blockT
    WkT = Wk.T
    WqT = Wq.T
    wkt = np.zeros((128, 512), f32)
    wqt = np.zeros((128, 512), f32)
    for k in range(2):
        for j in range(2):
            wkt[:, (k * 2 + j) * 128:(k * 2 + j + 1) * 128] = \
                WkT[k * 128:(k + 1) * 128, j * 128:(j + 1) * 128]
            wqt[:, (k * 2 + j) * 128:(k * 2 + j + 1) * 128] = \
                WqT[k * 128:(k + 1) * 128, j * 128:(j + 1) * 128]
    bkt = np.ascontiguousarray(bk.reshape(2, 128).T, dtype=f32)
    bqt = np.ascontiguousarray(bq.reshape(2, 128).T, dtype=f32)
    h0c = np.ascontiguousarray(hidden0.reshape(2, 128).T, dtype=f32)
    ones1 = np.ones((1, 128), f32)
    ident = np.eye(128, dtype=f32)
    tvs = (np.arange(128, dtype=f32)[:, None] +
           128.0 * np.arange(16, dtype=f32)[None, :]).astype(f32)
    w = dict(xb=xb, wstat=wstat, wkt=wkt, wqt=wqt, bkt=bkt, bqt=bqt,
             ones1=ones1, ident=ident, tvs=tvs, h0=h0c)
    if OUT_MODE == "tri":
        w["iota1"] = np.arange(N, dtype=f32)[None, :]
    return w


def _weights_key(inputs):
    h = hashlib.blake2b(digest_size=16)
    for k in ("decoder_init", "hidden0", "w_ih", "w_hh", "b_ih", "b_hh",
              "Wq", "bq", "Wk", "bk"):
        a = np.ascontiguousarray(np.asarray(inputs[k], dtype=np.float32))
        h.update(a.tobytes())
    return h.digest()


def _get_rt():
    if "rt" in _cache:
        return _cache["rt"]
    nc = _build()
    import concourse.mybir as mybir
    from concourse.bass2jax import (_bass_exec_p, install_neuronx_cc_hook,
                                    partition_id_tensor)
    import jax
    import jax.numpy as jnp
    from jax.sharding import Mesh, PartitionSpec, NamedSharding
    from jax.experimental.shard_map import shard_map

    install_neuronx_cc_hook()
    partition_name = nc.partition_id_tensor.name if nc.partition_id_tensor else None

    in_names = []
    out_names = []
    out_avals = []
    for alloc in nc.m.functions[0].allocations:
        if not isinstance(alloc, mybir.MemoryLocationSet):
            continue
        name = alloc.memorylocations[0].name
        if alloc.kind == "ExternalInput":
            if name != partition_name:
                in_names.append(name)
        elif alloc.kind == "ExternalOutput":
            out_names.append(name)
            out_avals.append(jax.core.ShapedArray(tuple(alloc.tensor_shape),
                                                  mybir.dt.np(alloc.dtype)))
    n_params = len(in_names)
    n_outs = len(out_avals)
    in_names_full = list(in_names) + out_names
    if partition_name is not None:
        in_names_full.append(partition_name)
    donate = tuple(range(n_params, n_params + n_outs))

    def _body(*args):
        operands = list(args)
        if partition_name is not None:
            operands.append(partition_id_tensor())
        return tuple(_bass_exec_p.bind(
            *operands, out_avals=tuple(out_avals), in_names=tuple(in_names_full),
            out_names=tuple(out_names), lowering_input_output_aliases=(),
            sim_require_finite=True, sim_require_nnan=True, nc=nc))

    devices = jax.devices()[:NCORES]
    gsz = NCORES // GROUPS
    groups = []
    in_specs = (PartitionSpec("core"),) * (n_params + n_outs)
    out_specs = (PartitionSpec("core"),) * n_outs
    for g in range(GROUPS):
        devs = devices[g * gsz:(g + 1) * gsz]
        mesh = Mesh(np.asarray(devs), ("core",))
        sh = NamedSharding(mesh, PartitionSpec("core"))
        sharded = jax.jit(
            shard_map(_body, mesh=mesh, in_specs=in_specs, out_specs=out_specs,
                      check_rep=False),
            donate_argnums=donate, keep_unused=True)
        zeros_jit = jax.jit(
            lambda gsz=gsz: tuple(
                jnp.zeros((gsz * av.shape[0], *av.shape[1:]), av.dtype)
                for av in out_avals),
            out_shardings=(sh,) * n_outs)
        groups.append(SimpleNamespace(devs=devs, sh=sh, sharded=sharded,
                                      zeros_jit=zeros_jit))

    rt = SimpleNamespace(nc=nc, jax=jax, in_names=in_names, out_names=out_names,
                         groups=groups, gsz=gsz,
                         weights_dev=None, weights_key=None,
                         pool=ThreadPoolExecutor(max_workers=32))
    _cache["rt"] = rt
    return rt


def _upload_weights(rt, inputs):
    key = _weights_key(inputs)
    if rt.weights_key == key and rt.weights_dev is not None:
        return rt.weights_dev
    w = _prep_weights(inputs)
    dev = []
    for grp in rt.groups:
        gd = {}
        for name, arr in w.items():
            t = np.tile(arr, (rt.gsz, 1))
            gd[name] = rt.jax.device_put(t, grp.sh)
        dev.append(gd)
    for gd in dev:
        rt.jax.block_until_ready(list(gd.values()))
    rt.weights_dev = dev
    rt.weights_key = key
    return dev


def _run(inputs, trace=False, tmpdir=None):
    rt = _get_rt()
    jax = rt.jax
    f32 = np.float32
    gsz = rt.gsz

    weights = _upload_weights(rt, inputs)

    node_embedding = np.asarray(inputs["node_embedding"], dtype=f32)
    z_g = np.asarray(inputs["z_g"], dtype=f32)
    c0_all = np.ascontiguousarray(
        z_g.reshape(NCORES, 2, 128).transpose(0, 2, 1)).reshape(NCORES * 128, 2)

    H = N // 2
    res = np.empty((B, N, N), f32)
    inv_scale = f32(1.0 / OUT_SCALE)
    recon_futs = []
    prev_nodes = None

    for g, grp in enumerate(rt.groups):
        if prev_nodes is not None:
            # stagger: keep the half-duplex link dedicated to the previous
            # group's upload; its exec+download then overlaps our upload
            jax.block_until_ready(prev_nodes)
        cores = list(range(g * gsz, (g + 1) * gsz))
        # per-call activations upload, shard-parallel (node split into two
        # tensors so 2*gsz transfers run concurrently)
        futs = {(b, j): rt.pool.submit(
                    jax.device_put, node_embedding[b, j * H:(j + 1) * H],
                    grp.devs[b - g * gsz])
                for b in cores for j in range(2)}
        zeros = grp.zeros_jit()
        node_arrs = [
            jax.make_array_from_single_device_arrays(
                (gsz * H, M), grp.sh, [futs[(b, j)].result() for b in cores])
            for j in range(2)]
        c0_g = c0_all[g * gsz * 128:(g + 1) * gsz * 128]
        per_call = {"node_a": node_arrs[0], "node_b": node_arrs[1], "c0": c0_g}
        args = [per_call[nm] if nm in per_call else weights[g][nm]
                for nm in rt.in_names]
        outs = grp.sharded(*args, *zeros)
        out_by_name = dict(zip(rt.out_names, outs))

        if OUT_MODE == "tri":
            ranks_fut = rt.pool.submit(np.asarray, out_by_name["outr"])

            def _recon(s, g=g, ranks_fut=ranks_fut):
                bl = s.index[0].start // RB
                buf = np.asarray(s.data)  # [RB, CW] u8
                rankn = ranks_fut.result()[bl * 128:(bl + 1) * 128]  # [128, 16]
                rank = rankn.T.reshape(N).astype(np.int64)  # rank[n]
                P = np.zeros((N, N), np.uint8)
                for i in range(NB):
                    P[i * RB:(i + 1) * RB, i * RB:] = buf[:, CO[i]:CO[i] + BW[i]]
                a = np.take(P, rank, axis=1)
                np.multiply(a, inv_scale, out=res[g * gsz + bl])

            recon_futs += [rt.pool.submit(_recon, s)
                           for s in out_by_name["outp"].addressable_shards]
        else:
            out_arr = out_by_name["out"]

            def _fetch(task, g=g):
                s, j = task
                bl = s.index[0].start // N
                a = np.asarray(s.data[j * H:(j + 1) * H])
                dst = res[g * gsz + bl, j * H:(j + 1) * H]
                if OUT_MODE == "u8":
                    np.multiply(a, inv_scale, out=dst)
                else:
                    np.copyto(dst, a, casting="unsafe")

            recon_futs += [rt.pool.submit(_fetch, (s, j))
                           for s in out_arr.addressable_shards for j in range(2)]
        prev_nodes = node_arrs

    for f in recon_futs:
        f.result()
    return res, SimpleNamespace(exec_time_ns=None, results=None)


def kernel(**inputs) -> np.ndarray:
    out, _ = _run(inputs, trace=False)
    return out


# revision 32
# speedup vs baseline: 1.1279x; 1.1047x over previous
import sys
import hashlib
from types import SimpleNamespace
from concurrent.futures import ThreadPoolExecutor

import numpy as np

sys.path.insert(0, "/opt/trn_rl_repo")

B, N, M = 8, 2048, 256
NCORES = 8
U = 64  # unroll factor inside hardware loops

# Output encoding:
#  "tri": permute probs columns by selection order on-device -> masked zeros
#         become a static lower triangle; ship only the packed upper triangle
#         (u8-quantized) + the rank vector, reconstruct on the host.
#  "u8":  full [N,N] probs matrix quantized to uint8.
#  probs are in [0,1]; code = round(p*OUT_SCALE) on the Act engine (round-to-
#  nearest cast), decode v/OUT_SCALE on host: quant err <= 0.5/253 ~ 2e-3.
OUT_MODE = "tri"  # "tri" | "u8" | "f16" | "f32"
OUT_SCALE = 253.0
# Pipeline groups: cores can be split into GROUPS sets run as separate
# staggered dispatches (group A's exec+download overlapping group B's upload).
# GROUPS=2 uses a backpressure stagger (blocking upload tasks on a capped
# pool) so no sync RPC sits on the critical path.
GROUPS = 2
# Block-packed triangle: each 32-row band [t0, t0+32) keeps columns [t0, N) —
# the static lower-triangle rest is exactly zero and is dropped. 2.13MB/core
# vs 4.2MB dense, with 64 rectangular DMAs per core.
RB = 32  # band height
NB = N // RB  # 64 bands
BW = [N - i * RB for i in range(NB)]  # band widths
CO = [sum(BW[:i]) for i in range(NB)]  # band column offsets in packed layout
CW = sum(BW)  # packed columns per 32-row band layer

_cache = {}


def _build():
    if "nc" in _cache:
        return _cache["nc"]
    from concourse import bass, tile, bacc
    import concourse.mybir as mybir

    fp32 = mybir.dt.float32
    u8 = mybir.dt.uint8
    odt = {"tri": u8, "u8": u8, "f16": mybir.dt.float16,
           "f32": fp32}[OUT_MODE]
    Alu = mybir.AluOpType
    Act = mybir.ActivationFunctionType
    AX = mybir.AxisListType
    ds = bass.ds

    nc = bacc.Bacc("TRN2", target_bir_lowering=False, debug=False,
                   num_devices=NCORES)

    node_a = nc.dram_tensor("node_a", [N // 2, M], fp32, kind="ExternalInput").ap()
    node_b = nc.dram_tensor("node_b", [N // 2, M], fp32, kind="ExternalInput").ap()
    c0_d = nc.dram_tensor("c0", [128, 2], fp32, kind="ExternalInput").ap()
    h0_d = nc.dram_tensor("h0", [128, 2], fp32, kind="ExternalInput").ap()
    xb_d = nc.dram_tensor("xb", [128, 8], fp32, kind="ExternalInput").ap()
    wstat_d = nc.dram_tensor("wstat", [128, 2048], fp32, kind="ExternalInput").ap()
    wkt_d = nc.dram_tensor("wkt", [128, 512], fp32, kind="ExternalInput").ap()
    wqt_d = nc.dram_tensor("wqt", [128, 512], fp32, kind="ExternalInput").ap()
    bkt_d = nc.dram_tensor("bkt", [128, 2], fp32, kind="ExternalInput").ap()
    bqt_d = nc.dram_tensor("bqt", [128, 2], fp32, kind="ExternalInput").ap()
    ones_d = nc.dram_tensor("ones1", [1, 128], fp32, kind="ExternalInput").ap()
    id_d = nc.dram_tensor("ident", [128, 128], fp32, kind="ExternalInput").ap()
    tvs_d = nc.dram_tensor("tvs", [128, 16], fp32, kind="ExternalInput").ap()
    if OUT_MODE == "tri":
        iota1_d = nc.dram_tensor("iota1", [1, N], fp32, kind="ExternalInput").ap()
        outp_d = nc.dram_tensor("outp", [RB, CW], u8, kind="ExternalOutput").ap()
        outr_d = nc.dram_tensor("outr", [128, 16], fp32, kind="ExternalOutput").ap()
        out_d = None
    else:
        out_d = nc.dram_tensor("out", [N, N], odt, kind="ExternalOutput").ap()

    def emit_pass(tc, rep):
        sx = f"_r{rep}"
        # ---- constants in SBUF ----
        xb_sb, f_xb = tc.tile([128, 8], fp32, name="xb_sb" + sx)
        wstat_sb, f_wstat = tc.tile([128, 2048], fp32, name="wstat_sb" + sx)
        wkt_sb, f_wkt = tc.tile([128, 512], fp32, name="wkt_sb" + sx)
        wqt_sb, f_wqt = tc.tile([128, 512], fp32, name="wqt_sb" + sx)
        bkt_sb, f_bkt = tc.tile([128, 2], fp32, name="bkt_sb" + sx)
        bqt_sb, f_bqt = tc.tile([128, 2], fp32, name="bqt_sb" + sx)
        ones_sb, f_ones = tc.tile([1, 128], fp32, name="ones_sb" + sx)
        id_sb, f_id = tc.tile([128, 128], fp32, name="id_sb" + sx)
        tvs_sb, f_tvs = tc.tile([128, 16], fp32, name="tvs_sb" + sx)
        for sb, dr in ((xb_sb, xb_d), (wstat_sb, wstat_d), (wkt_sb, wkt_d),
                       (wqt_sb, wqt_d), (bkt_sb, bkt_d), (bqt_sb, bqt_d),
                       (ones_sb, ones_d), (id_sb, id_d), (tvs_sb, tvs_d)):
            nc.gpsimd.dma_start(sb[:], dr[:, :])

        # main PSUM pool used across all phases
        p512_cm = tc.tile_pool(name="p512" + sx, bufs=2, space="PSUM")
        p512 = p512_cm.__enter__()

        # ---- persistent big tensors ----
        # Alloc order is load-bearing: the tile allocator releases strictly
        # LIFO, so tiles that die earliest are allocated last.
        QT, f_QT = tc.tile([128, 2, N], fp32, name="QT" + sx)
        if OUT_MODE == "tri":
            KpT, f_KpT = tc.tile([128, 2, N], fp32, name="KpT" + sx)
            iotab, f_iotab = tc.tile([128, N], fp32, name="iotab" + sx)
            iota_sb, f_iota = tc.tile([1, N], fp32, name="iota_sb" + sx)
            keysN, f_keysN = tc.tile([128, 16, 256], fp32, name="keysN" + sx)
        keysT, f_keysT = tc.tile([128, 2, N], fp32, name="keysT" + sx)

        # ---- prologue: node -> nodeT -> keysT (+ keysN for tri) ----
        nodeN, f_nodeN = tc.tile([128, 16, 256], fp32, name="nodeN" + sx)
        nodeT, f_nodeT = tc.tile([128, 2, N], fp32, name="nodeT" + sx)
        for c in range(16):
            src = node_a if c < 8 else node_b
            c0_ = c if c < 8 else c - 8
            nc.gpsimd.dma_start(nodeN[:, c, :], src[c0_ * 128:(c0_ + 1) * 128, :])
        ptp_cm = tc.tile_pool(name="ptp" + sx, bufs=2, space="PSUM")
        ptp = ptp_cm.__enter__()
        for c in range(16):
            for k in range(2):
                pt = ptp.tile([128, 128], fp32)
                nc.tensor.transpose(pt[:], nodeN[:, c, k * 128:(k + 1) * 128], id_sb[:])
                nc.scalar.activation(nodeT[:, k, c * 128:(c + 1) * 128], pt[:], Act.Copy)
        ptp_cm.__exit__(None, None, None)
        for j2 in range(2):
            for nb in range(4):
                pk = p512.tile([128, 512], fp32, tag="ps")
                for k in range(2):
                    nc.tensor.matmul(pk[:], wkt_sb[:, (k * 2 + j2) * 128:(k * 2 + j2 + 1) * 128],
                                     nodeT[:, k, nb * 512:(nb + 1) * 512],
                                     start=(k == 0), stop=(k == 1))
                nc.vector.tensor_scalar(out=keysT[:, j2, nb * 512:(nb + 1) * 512],
                                        in0=pk[:], scalar1=bkt_sb[:, j2:j2 + 1],
                                        scalar2=None, op0=Alu.add)
        if OUT_MODE == "tri":
            # keysN[n(p), m(f)] = (node @ Wk^T), unbiased, n-on-partition.
            # Used later to gather keys by selection order via a one-hot matmul.
            for c in range(16):
                pn = p512.tile([128, 256], fp32, tag="ps")
                for kc in range(2):
                    nc.tensor.matmul(pn[:], nodeT[:, kc, c * 128:(c + 1) * 128],
                                     wkt_sb[:, kc * 256:(kc + 1) * 256],
                                     start=(kc == 0), stop=(kc == 1))
                nc.scalar.activation(keysN[:, c, :], pn[:], Act.Copy)
        f_nodeT()
        f_nodeN()

        # ---- phase 1: LSTM unroll -> HTx ----
        HTx, f_HTx = tc.tile([128, 2, N + 1], fp32, name="HTx" + sx)
        ct, f_ct = tc.tile([128, 2], fp32, name="ct" + sx)
        gsb, f_gsb = tc.tile([128, 8], fp32, name="gsb" + sx)
        sfo, f_sfo = tc.tile([128, 6], fp32, name="sfo" + sx)
        gt, f_gt = tc.tile([128, 2], fp32, name="gt" + sx)
        t1, f_t1 = tc.tile([128, 2], fp32, name="t1" + sx)
        tct, f_tct = tc.tile([128, 2], fp32, name="tct" + sx)
        nc.gpsimd.dma_start(HTx[:, 0, 0:1], h0_d[:, 0:1])
        nc.gpsimd.dma_start(HTx[:, 1, 0:1], h0_d[:, 1:2])
        nc.gpsimd.dma_start(ct[:], c0_d[:, :])
        gpp_cm = tc.tile_pool(name="gpp" + sx, bufs=2, space="PSUM")
        gpp = gpp_cm.__enter__()
        with tc.For_i(0, N, step=U) as iv:
            for u in range(U):
                t = iv + u
                gp = gpp.tile([128, 8], fp32)
                for m2 in range(8):
                    nc.tensor.matmul(gp[:, m2:m2 + 1],
                                     wstat_sb[:, (m2 * 2) * 128:(m2 * 2 + 1) * 128],
                                     HTx[:, 0, ds(t, 1)], start=True, stop=False)
                    nc.tensor.matmul(gp[:, m2:m2 + 1],
                                     wstat_sb[:, (m2 * 2 + 1) * 128:(m2 * 2 + 2) * 128],
                                     HTx[:, 1, ds(t, 1)], start=False, stop=True)
                nc.vector.tensor_tensor(out=gsb[:], in0=gp[:], in1=xb_sb[:], op=Alu.add)
                nc.scalar.activation(sfo[:], gsb[:, 0:6], Act.Sigmoid)
                nc.scalar.activation(gt[:], gsb[:, 6:8], Act.Tanh)
                nc.vector.tensor_tensor(out=t1[:], in0=sfo[:, 0:2], in1=gt[:], op=Alu.mult)
                nc.vector.tensor_tensor(out=ct[:], in0=sfo[:, 2:4], in1=ct[:], op=Alu.mult)
                nc.vector.tensor_tensor(out=ct[:], in0=ct[:], in1=t1[:], op=Alu.add)
                nc.scalar.activation(tct[:], ct[:], Act.Tanh)
                nc.vector.tensor_tensor(out=HTx[:, 0, ds(t + 1, 1)],
                                        in0=sfo[:, 4:5], in1=tct[:, 0:1], op=Alu.mult)
                nc.vector.tensor_tensor(out=HTx[:, 1, ds(t + 1, 1)],
                                        in0=sfo[:, 5:6], in1=tct[:, 1:2], op=Alu.mult)
        gpp_cm.__exit__(None, None, None)

        # ---- QT = Wq @ h + bq (feature-on-partition) ----
        for j2 in range(2):
            for tb in range(4):
                pq = p512.tile([128, 512], fp32, tag="ps")
                for k in range(2):
                    nc.tensor.matmul(pq[:], wqt_sb[:, (k * 2 + j2) * 128:(k * 2 + j2 + 1) * 128],
                                     HTx[:, k, 1 + tb * 512:1 + (tb + 1) * 512],
                                     start=(k == 0), stop=(k == 1))
                nc.vector.tensor_scalar(out=QT[:, j2, tb * 512:(tb + 1) * 512],
                                        in0=pq[:], scalar1=bqt_sb[:, j2:j2 + 1],
                                        scalar2=None, op0=Alu.add)
        f_tct(); f_t1(); f_gt(); f_sfo(); f_gsb(); f_ct(); f_HTx()

        # ---- phase 2+3 interleaved: ST blocks + argmax-rank chain ----
        if OUT_MODE != "tri":
            rb, f_maskb = tc.tile([128, N], fp32, name="rb" + sx)
            trs, f_trs = tc.tile([16, 128], fp32, name="trs" + sx)
        rankn, f_rankn = tc.tile([128, 16], fp32, name="rankn" + sx)
        if OUT_MODE != "tri":
            rr, f_rr = tc.tile([1, N], fp32, name="rr" + sx)
        stp_cm = tc.tile_pool(name="stp" + sx, bufs=2)
        stp = stp_cm.__enter__()
        ma, f_ma = tc.tile([128, 16], fp32, name="ma" + sx)
        ms, f_ms = tc.tile([128, 16], fp32, name="ms" + sx)
        mk, f_mk = tc.tile([128, 16], fp32, name="mk" + sx)
        pm, f_pm = tc.tile([128, 1], fp32, name="pm" + sx)
        gm, f_gm = tc.tile([1, 1], fp32, name="gm" + sx)
        dl, f_dl = tc.tile([128, 16], fp32, name="dl" + sx)
        tpp_cm = tc.tile_pool(name="tpp" + sx, bufs=2, space="PSUM")
        tpp = tpp_cm.__enter__()
        gbp_cm = tc.tile_pool(name="gbp" + sx, bufs=2, space="PSUM")
        gbp = gbp_cm.__enter__()
        nc.vector.memset(ma[:], 0.0)
        nc.vector.memset(ms[:], 0.0)

        def emit_st_block(tb):
            st_tb = stp.tile([128, 16, 512], fp32, name=f"st{tb}" + sx, tag="st")
            for c in range(16):
                pS = p512.tile([128, 512], fp32, tag="ps")
                for k in range(2):
                    nc.tensor.matmul(pS[:], keysT[:, k, c * 128:(c + 1) * 128],
                                     QT[:, k, tb * 512:(tb + 1) * 512],
                                     start=(k == 0), stop=(k == 1))
                nc.scalar.activation(st_tb[:, c, :], pS[:], Act.Copy)
            return st_tb

        def emit_l3(st_tb):
            with tc.For_i(0, 512, step=U) as iv:
                for u in range(U):
                    tl_ = iv + u
                    nc.vector.tensor_tensor(out=mk[:], in0=st_tb[:, :, ds(tl_, 1)],
                                            in1=ma[:], op=Alu.add)
                    # ms += 1 for already-selected boxes (exact small ints)
                    nc.vector.scalar_tensor_tensor(out=ms[:], in0=ma[:],
                                                   scalar=-(2.0 ** -30),
                                                   in1=ms[:], op0=Alu.mult,
                                                   op1=Alu.add)
                    nc.vector.reduce_max(out=pm[:], in_=mk[:], axis=AX.X)
                    tp = tpp.tile([1, 128], fp32, tag="tp")
                    nc.tensor.transpose(tp[:], pm[:], id_sb[:])
                    nc.vector.reduce_max(out=gm[:], in_=tp[:], axis=AX.X)
                    gb = gbp.tile([128, 1], fp32)
                    nc.tensor.matmul(gb[:], ones_sb[:], gm[:], start=True, stop=True)
                    nc.vector.tensor_scalar(out=dl[:], in0=mk[:], scalar1=gb[:],
                                            scalar2=-(2.0 ** 30), op0=Alu.is_ge,
                                            op1=Alu.mult)
                    nc.vector.tensor_tensor(out=ma[:], in0=ma[:], in1=dl[:], op=Alu.add)

        blocks = [emit_st_block(0), emit_st_block(1)]
        emit_l3(blocks[0])
        blocks.append(emit_st_block(2))
        emit_l3(blocks[1])
        blocks.append(emit_st_block(3))
        emit_l3(blocks[2])
        emit_l3(blocks[3])

        # rank_n = 2047 - ms_n (exact small integers)
        nc.vector.tensor_scalar(out=rankn[:], in0=ms[:], scalar1=-1.0,
                                scalar2=2047.0, op0=Alu.mult, op1=Alu.add)

        if OUT_MODE == "tri":
            nc.gpsimd.dma_start(outr_d[:, :], rankn[:])
            # iotab[p, r] = r (broadcast of the column-index row)
            nc.gpsimd.dma_start(iota_sb[:], iota1_d[:, :])
            for g in range(4):
                pr = p512.tile([128, 512], fp32, tag="ps")
                nc.tensor.matmul(pr[:], ones_sb[:], iota_sb[0:1, g * 512:(g + 1) * 512],
                                 start=True, stop=True)
                nc.scalar.activation(iotab[:, g * 512:(g + 1) * 512], pr[:], Act.Copy)
            # K_permT[m, r] = keys[pi(r), m] + bk[m] via one-hot gather matmul:
            # Pi[n, r] = 1{rank[n] == r}; one-hot fp32 matmul copies values exactly.
            pip_cm = tc.tile_pool(name="pip" + sx, bufs=3)
            pip = pip_cm.__enter__()
            for j2 in range(2):
                for rg in range(4):
                    pq = p512.tile([128, 512], fp32, tag="ps")
                    for c in range(16):
                        pi = pip.tile([128, 512], fp32, tag="pi")
                        nc.vector.tensor_scalar(out=pi[:],
                                                in0=iotab[:, rg * 512:(rg + 1) * 512],
                                                scalar1=rankn[:, c:c + 1],
                                                scalar2=None, op0=Alu.is_equal)
                        nc.tensor.matmul(pq[:], keysN[:, c, j2 * 128:(j2 + 1) * 128],
                                         pi[:], start=(c == 0), stop=(c == 15))
                    nc.vector.tensor_scalar(out=KpT[:, j2, rg * 512:(rg + 1) * 512],
                                            in0=pq[:], scalar1=bkt_sb[:, j2:j2 + 1],
                                            scalar2=None, op0=Alu.add)
            pip_cm.__exit__(None, None, None)
            keysP4 = KpT
        else:
            # broadcast rank over rows: rb[p, n] = rank[n]
            tp2 = tpp.tile([16, 128], fp32, tag="tp")
            nc.tensor.transpose(tp2[:], rankn[:], id_sb[:])
            nc.scalar.activation(trs[:], tp2[:], Act.Copy)
            for c in range(16):
                nc.gpsimd.dma_start(rr[0:1, c * 128:(c + 1) * 128], trs[c:c + 1, :])
            for g in range(4):
                pr = p512.tile([128, 512], fp32, tag="ps")
                nc.tensor.matmul(pr[:], ones_sb[:], rr[0:1, g * 512:(g + 1) * 512],
                                 start=True, stop=True)
                nc.scalar.activation(rb[:, g * 512:(g + 1) * 512], pr[:], Act.Copy)
            iotab = rb
            keysP4 = keysT

        gbp_cm.__exit__(None, None, None)
        tpp_cm.__exit__(None, None, None)
        f_dl(); f_gm(); f_pm(); f_mk(); f_ms(); f_ma()
        stp_cm.__exit__(None, None, None)
        if OUT_MODE == "tri":
            f_rankn()
            f_keysT()
            f_keysN()
            f_iota()

        # ---- phase 4: probs rows, masked softmax, DMA out ----
        # mask = (iotab >= t): original column order compares rank[n] >= t;
        # permuted order compares column index r >= t (static triangle).
        rs4, f_rs4 = tc.tile([128, 4], fp32, name="rs4" + sx)
        rsum, f_rsum = tc.tile([128, 1], fp32, name="rsum" + sx)
        rinv, f_rinv = tc.tile([128, 1], fp32, name="rinv" + sx)
        rsc, f_rsc = tc.tile([128, 1], fp32, name="rsc" + sx)
        esp_cm = tc.tile_pool(name="esp" + sx, bufs=2)
        esp = esp_cm.__enter__()
        eop_cm = tc.tile_pool(name="eop" + sx, bufs=2)
        eop = eop_cm.__enter__()
        for blk in range(16):
            es = esp.tile([128, N], fp32, name=f"es{blk}" + sx, tag="es")
            for nb in range(4):
                pS = p512.tile([128, 512], fp32, tag="ps")
                for k in range(2):
                    nc.tensor.matmul(pS[:], QT[:, k, blk * 128:(blk + 1) * 128],
                                     keysP4[:, k, nb * 512:(nb + 1) * 512],
                                     start=(k == 0), stop=(k == 1))
                nc.scalar.activation(es[:, nb * 512:(nb + 1) * 512], pS[:], Act.Exp)
                nc.vector.scalar_tensor_tensor(
                    out=es[:, nb * 512:(nb + 1) * 512],
                    in0=iotab[:, nb * 512:(nb + 1) * 512],
                    scalar=tvs_sb[:, blk:blk + 1],
                    in1=es[:, nb * 512:(nb + 1) * 512],
                    op0=Alu.is_ge, op1=Alu.mult,
                    accum_out=rs4[:, nb:nb + 1])
            nc.vector.reduce_sum(out=rsum[:], in_=rs4[:], axis=AX.X)
            nc.vector.reciprocal(rinv[:], rsum[:])
            eo = eop.tile([128, N], odt, name=f"eo{blk}" + sx, tag="eo")
            if OUT_MODE in ("tri", "u8"):
                nc.vector.tensor_scalar(out=rsc[:], in0=rinv[:], scalar1=OUT_SCALE,
                                        scalar2=None, op0=Alu.mult)
                for nb in range(4):
                    nc.scalar.activation(eo[:, nb * 512:(nb + 1) * 512],
                                         es[:, nb * 512:(nb + 1) * 512],
                                         Act.Copy, scale=rsc[:])
            else:
                for nb in range(4):
                    nc.scalar.activation(eo[:, nb * 512:(nb + 1) * 512],
                                         es[:, nb * 512:(nb + 1) * 512],
                                         Act.Copy, scale=rinv[:])
            if OUT_MODE == "tri":
                for sub in range(128 // RB):
                    i = blk * (128 // RB) + sub  # global band index
                    nc.gpsimd.dma_start(outp_d[:, CO[i]:CO[i] + BW[i]],
                                        eo[sub * RB:(sub + 1) * RB, i * RB:N])
            else:
                nc.gpsimd.dma_start(out_d[blk * 128:(blk + 1) * 128, :], eo[:])
        eop_cm.__exit__(None, None, None)
        esp_cm.__exit__(None, None, None)
        f_rsc(); f_rinv(); f_rsum(); f_rs4()
        if OUT_MODE == "tri":
            f_iotab(); f_KpT(); f_QT()
        else:
            f_rr(); f_rankn(); f_trs(); f_maskb()
            f_keysT(); f_QT()
        p512_cm.__exit__(None, None, None)
        f_tvs(); f_id(); f_ones(); f_bqt(); f_bkt(); f_wqt(); f_wkt(); f_wstat(); f_xb()

    from concourse import tile
    with tile.TileContext(nc) as tc:
        emit_pass(tc, 0)

    nc.compile()
    _cache["nc"] = nc
    return nc


def _prep_weights(inputs):
    """Per-core-identical operands, keyed by dram tensor name (per-core shapes)."""
    f32 = np.float32
    decoder_init = np.asarray(inputs["decoder_init"], dtype=f32)
    hidden0 = np.asarray(inputs["hidden0"], dtype=f32)
    w_ih = np.asarray(inputs["w_ih"], dtype=f32)
    w_hh = np.asarray(inputs["w_hh"], dtype=f32)
    b_ih = np.asarray(inputs["b_ih"], dtype=f32)
    b_hh = np.asarray(inputs["b_hh"], dtype=f32)
    Wq = np.asarray(inputs["Wq"], dtype=f32)
    bq = np.asarray(inputs["bq"], dtype=f32)
    Wk = np.asarray(inputs["Wk"], dtype=f32)
    bk = np.asarray(inputs["bk"], dtype=f32)

    perm = np.concatenate([np.arange(0, 256), np.arange(256, 512),
                           np.arange(768, 1024), np.arange(512, 768)])
    w_hh_p = w_hh[perm]
    x_proj = decoder_init @ w_ih.T + b_ih
    xb = np.ascontiguousarray(((x_proj + b_hh)[perm]).reshape(8, 128).T, dtype=f32)
    wstat = np.zeros((128, 2048), f32)
    for m2 in range(8):
        for k in range(2):
            blockT = w_hh_p[m2 * 128:(m2 + 1) * 128, k * 128:(k + 1) * 128].T
            wstat[:, (m2 * 2 + k) * 128:(m2 * 2 + k + 1) * 128] = blockT
    WkT = Wk.T
    WqT = Wq.T
    wkt = np.zeros((128, 512), f32)
    wqt = np.zeros((128, 512), f32)
    for k in range(2):
        for j in range(2):
            wkt[:, (k * 2 + j) * 128:(k * 2 + j + 1) * 128] = \
                WkT[k * 128:(k + 1) * 128, j * 128:(j + 1) * 128]
            wqt[:, (k * 2 + j) * 128:(k * 2 + j + 1) * 128] = \
                WqT[k * 128:(k + 1) * 128, j * 128:(j + 1) * 128]
    bkt = np.ascontiguousarray(bk.reshape(2, 128).T, dtype=f32)
    bqt = np.ascontiguousarray(bq.reshape(2, 128).T, dtype=f32)
    h0c = np.ascontiguousarray(hidden0.reshape(2, 128).T, dtype=f32)
    ones1 = np.ones((1, 128), f32)
    ident = np.eye(128, dtype=f32)
    tvs = (np.arange(128, dtype=f32)[:, None] +
           128.0 * np.arange(16, dtype=f32)[None, :]).astype(f32)
    w = dict(xb=xb, wstat=wstat, wkt=wkt, wqt=wqt, bkt=bkt, bqt=bqt,
             ones1=ones1, ident=ident, tvs=tvs, h0=h0c)
    if OUT_MODE == "tri":
        w["iota1"] = np.arange(N, dtype=f32)[None, :]
    return w


def _weights_key(inputs):
    h = hashlib.blake2b(digest_size=16)
    for k in ("decoder_init", "hidden0", "w_ih", "w_hh", "b_ih", "b_hh",
              "Wq", "bq", "Wk", "bk"):
        a = np.ascontiguousarray(np.asarray(inputs[k], dtype=np.float32))
        h.update(a.tobytes())
    return h.digest()


def _get_rt():
    if "rt" in _cache:
        return _cache["rt"]
    nc = _build()
    import concourse.mybir as mybir
    from concourse.bass2jax import (_bass_exec_p, install_neuronx_cc_hook,
                                    partition_id_tensor)
    import jax
    import jax.numpy as jnp
    from jax.sharding import Mesh, PartitionSpec, NamedSharding
    from jax.experimental.shard_map import shard_map

    install_neuronx_cc_hook()
    partition_name = nc.partition_id_tensor.name if nc.partition_id_tensor else None

    in_names = []
    out_names = []
    out_avals = []
    for alloc in nc.m.functions[0].allocations:
        if not isinstance(alloc, mybir.MemoryLocationSet):
            continue
        name = alloc.memorylocations[0].name
        if alloc.kind == "ExternalInput":
            if name != partition_name:
                in_names.append(name)
        elif alloc.kind == "ExternalOutput":
            out_names.append(name)
            out_avals.append(jax.core.ShapedArray(tuple(alloc.tensor_shape),
                                                  mybir.dt.np(alloc.dtype)))
    n_params = len(in_names)
    n_outs = len(out_avals)
    in_names_full = list(in_names) + out_names
    if partition_name is not None:
        in_names_full.append(partition_name)
    donate = tuple(range(n_params, n_params + n_outs))

    def _body(*args):
        operands = list(args)
        if partition_name is not None:
            operands.append(partition_id_tensor())
        return tuple(_bass_exec_p.bind(
            *operands, out_avals=tuple(out_avals), in_names=tuple(in_names_full),
            out_names=tuple(out_names), lowering_input_output_aliases=(),
            sim_require_finite=True, sim_require_nnan=True, nc=nc))

    devices = jax.devices()[:NCORES]
    gsz = NCORES // GROUPS
    groups = []
    in_specs = (PartitionSpec("core"),) * (n_params + n_outs)
    out_specs = (PartitionSpec("core"),) * n_outs
    for g in range(GROUPS):
        devs = devices[g * gsz:(g + 1) * gsz]
        mesh = Mesh(np.asarray(devs), ("core",))
        sh = NamedSharding(mesh, PartitionSpec("core"))
        sharded = jax.jit(
            shard_map(_body, mesh=mesh, in_specs=in_specs, out_specs=out_specs,
                      check_rep=False),
            donate_argnums=donate, keep_unused=True)
        zeros_jit = jax.jit(
            lambda gsz=gsz: tuple(
                jnp.zeros((gsz * av.shape[0], *av.shape[1:]), av.dtype)
                for av in out_avals),
            out_shardings=(sh,) * n_outs)
        groups.append(SimpleNamespace(devs=devs, sh=sh, sharded=sharded,
                                      zeros_jit=zeros_jit))

    rt = SimpleNamespace(nc=nc, jax=jax, in_names=in_names, out_names=out_names,
                         groups=groups, gsz=gsz,
                         weights_dev=None, weights_key=None,
                         pool=ThreadPoolExecutor(max_workers=32),
                         up_pool=ThreadPoolExecutor(max_workers=2 * gsz))
    _cache["rt"] = rt
    return rt


def _upload_weights(rt, inputs):
    key = _weights_key(inputs)
    if rt.weights_key == key and rt.weights_dev is not None:
        return rt.weights_dev
    w = _prep_weights(inputs)
    dev = []
    for grp in rt.groups:
        gd = {}
        for name, arr in w.items():
            t = np.tile(arr, (rt.gsz, 1))
            gd[name] = rt.jax.device_put(t, grp.sh)
        dev.append(gd)
    for gd in dev:
        rt.jax.block_until_ready(list(gd.values()))
    rt.weights_dev = dev
    rt.weights_key = key
    return dev


def _run(inputs, trace=False, tmpdir=None):
    rt = _get_rt()
    jax = rt.jax
    f32 = np.float32
    gsz = rt.gsz

    weights = _upload_weights(rt, inputs)

    node_embedding = np.asarray(inputs["node_embedding"], dtype=f32)
    z_g = np.asarray(inputs["z_g"], dtype=f32)
    c0_all = np.ascontiguousarray(
        z_g.reshape(NCORES, 2, 128).transpose(0, 2, 1)).reshape(NCORES * 128, 2)

    H = N // 2
    res = np.empty((B, N, N), f32)
    inv_scale = f32(1.0 / OUT_SCALE)
    recon_futs = []
    prev_nodes = None

    # All groups' upload tasks are queued up front on a worker pool sized to
    # one group's chunk count: group g+1's transfers begin exactly as group
    # g's workers free up (backpressure stagger, no sync RPC). The tasks
    # block until their bytes land so pool slots model in-flight transfers.
    def _up_block(b, j, grp):
        a = jax.device_put(node_embedding[b, j * H:(j + 1) * H],
                           grp.devs[b % gsz])
        jax.block_until_ready(a)
        return a

    if GROUPS > 1:
        up_pool = rt.up_pool
        all_futs = {(b, j): up_pool.submit(_up_block, b, j, rt.groups[b // gsz])
                    for b in range(NCORES) for j in range(2)}

    for g, grp in enumerate(rt.groups):
        cores = list(range(g * gsz, (g + 1) * gsz))
        # per-call activations upload, shard-parallel (node split into two
        # tensors so 2*gsz transfers run concurrently)
        if GROUPS > 1:
            futs = all_futs
        else:
            futs = {(b, j): rt.pool.submit(
                        jax.device_put, node_embedding[b, j * H:(j + 1) * H],
                        grp.devs[b - g * gsz])
                    for b in cores for j in range(2)}
        zeros = grp.zeros_jit()
        node_arrs = [
            jax.make_array_from_single_device_arrays(
                (gsz * H, M), grp.sh, [futs[(b, j)].result() for b in cores])
            for j in range(2)]
        c0_g = c0_all[g * gsz * 128:(g + 1) * gsz * 128]
        per_call = {"node_a": node_arrs[0], "node_b": node_arrs[1], "c0": c0_g}
        args = [per_call[nm] if nm in per_call else weights[g][nm]
                for nm in rt.in_names]
        outs = grp.sharded(*args, *zeros)
        out_by_name = dict(zip(rt.out_names, outs))

        if OUT_MODE == "tri":
            ranks_fut = rt.pool.submit(np.asarray, out_by_name["outr"])

            def _recon(s, g=g, ranks_fut=ranks_fut):
                bl = s.index[0].start // RB
                buf = np.asarray(s.data)  # [RB, CW] u8
                rankn = ranks_fut.result()[bl * 128:(bl + 1) * 128]  # [128, 16]
                rank = rankn.T.reshape(N).astype(np.int64)  # rank[n]
                P = np.zeros((N, N), np.uint8)
                for i in range(NB):
                    P[i * RB:(i + 1) * RB, i * RB:] = buf[:, CO[i]:CO[i] + BW[i]]
                a = np.take(P, rank, axis=1)
                np.multiply(a, inv_scale, out=res[g * gsz + bl])

            recon_futs += [rt.pool.submit(_recon, s)
                           for s in out_by_name["outp"].addressable_shards]
        else:
            out_arr = out_by_name["out"]

            def _fetch(task, g=g):
                s, j = task
                bl = s.index[0].start // N
                a = np.asarray(s.data[j * H:(j + 1) * H])
                dst = res[g * gsz + bl, j * H:(j + 1) * H]
                if OUT_MODE == "u8":
                    np.multiply(a, inv_scale, out=dst)
                else:
                    np.copyto(dst, a, casting="unsafe")

            recon_futs += [rt.pool.submit(_fetch, (s, j))
                           for s in out_arr.addressable_shards for j in range(2)]
        prev_nodes = node_arrs

    for f in recon_futs:
        f.result()
    return res, SimpleNamespace(exec_time_ns=None, results=None)


def kernel(**inputs) -> np.ndarray:
    out, _ = _run(inputs, trace=False)
    return out


# revision 33
# speedup vs baseline: 1.1381x; 1.0090x over previous
import sys
import hashlib
from types import SimpleNamespace
from concurrent.futures import ThreadPoolExecutor

import numpy as np

sys.path.insert(0, "/opt/trn_rl_repo")

B, N, M = 8, 2048, 256
NCORES = 8
U = 64  # unroll factor inside hardware loops

# Output encoding:
#  "tri": permute probs columns by selection order on-device -> masked zeros
#         become a static lower triangle; ship only the packed upper triangle
#         (u8-quantized) + the rank vector, reconstruct on the host.
#  "u8":  full [N,N] probs matrix quantized to uint8.
#  probs are in [0,1]; code = round(p*OUT_SCALE) on the Act engine (round-to-
#  nearest cast), decode v/OUT_SCALE on host: quant err <= 0.5/253 ~ 2e-3.
OUT_MODE = "tri"  # "tri" | "u8" | "f16" | "f32"
OUT_SCALE = 253.0
# Pipeline groups: cores can be split into GROUPS sets run as separate
# staggered dispatches (group A's exec+download overlapping group B's upload).
# Measured: GROUPS=2 (both sync-RPC and backpressure stagger variants) is
# within noise of GROUPS=1 — best-case runs sit at the byte floor either way.
# Keep the simpler single-dispatch path.
GROUPS = 1
# Block-packed triangle: each 32-row band [t0, t0+32) keeps columns [t0, N) —
# the static lower-triangle rest is exactly zero and is dropped. 2.13MB/core
# vs 4.2MB dense, with 64 rectangular DMAs per core.
RB = 32  # band height
NB = N // RB  # 64 bands
BW = [N - i * RB for i in range(NB)]  # band widths
CO = [sum(BW[:i]) for i in range(NB)]  # band column offsets in packed layout
CW = sum(BW)  # packed columns per 32-row band layer

_cache = {}


def _build():
    if "nc" in _cache:
        return _cache["nc"]
    from concourse import bass, tile, bacc
    import concourse.mybir as mybir

    fp32 = mybir.dt.float32
    u8 = mybir.dt.uint8
    odt = {"tri": u8, "u8": u8, "f16": mybir.dt.float16,
           "f32": fp32}[OUT_MODE]
    Alu = mybir.AluOpType
    Act = mybir.ActivationFunctionType
    AX = mybir.AxisListType
    ds = bass.ds

    nc = bacc.Bacc("TRN2", target_bir_lowering=False, debug=False,
                   num_devices=NCORES)

    node_a = nc.dram_tensor("node_a", [N // 2, M], fp32, kind="ExternalInput").ap()
    node_b = nc.dram_tensor("node_b", [N // 2, M], fp32, kind="ExternalInput").ap()
    c0_d = nc.dram_tensor("c0", [128, 2], fp32, kind="ExternalInput").ap()
    h0_d = nc.dram_tensor("h0", [128, 2], fp32, kind="ExternalInput").ap()
    xb_d = nc.dram_tensor("xb", [128, 8], fp32, kind="ExternalInput").ap()
    wstat_d = nc.dram_tensor("wstat", [128, 2048], fp32, kind="ExternalInput").ap()
    wkt_d = nc.dram_tensor("wkt", [128, 512], fp32, kind="ExternalInput").ap()
    wqt_d = nc.dram_tensor("wqt", [128, 512], fp32, kind="ExternalInput").ap()
    bkt_d = nc.dram_tensor("bkt", [128, 2], fp32, kind="ExternalInput").ap()
    bqt_d = nc.dram_tensor("bqt", [128, 2], fp32, kind="ExternalInput").ap()
    ones_d = nc.dram_tensor("ones1", [1, 128], fp32, kind="ExternalInput").ap()
    id_d = nc.dram_tensor("ident", [128, 128], fp32, kind="ExternalInput").ap()
    tvs_d = nc.dram_tensor("tvs", [128, 16], fp32, kind="ExternalInput").ap()
    if OUT_MODE == "tri":
        iota1_d = nc.dram_tensor("iota1", [1, N], fp32, kind="ExternalInput").ap()
        outp_d = nc.dram_tensor("outp", [RB, CW], u8, kind="ExternalOutput").ap()
        outr_d = nc.dram_tensor("outr", [128, 16], fp32, kind="ExternalOutput").ap()
        out_d = None
    else:
        out_d = nc.dram_tensor("out", [N, N], odt, kind="ExternalOutput").ap()

    def emit_pass(tc, rep):
        sx = f"_r{rep}"
        # ---- constants in SBUF ----
        xb_sb, f_xb = tc.tile([128, 8], fp32, name="xb_sb" + sx)
        wstat_sb, f_wstat = tc.tile([128, 2048], fp32, name="wstat_sb" + sx)
        wkt_sb, f_wkt = tc.tile([128, 512], fp32, name="wkt_sb" + sx)
        wqt_sb, f_wqt = tc.tile([128, 512], fp32, name="wqt_sb" + sx)
        bkt_sb, f_bkt = tc.tile([128, 2], fp32, name="bkt_sb" + sx)
        bqt_sb, f_bqt = tc.tile([128, 2], fp32, name="bqt_sb" + sx)
        ones_sb, f_ones = tc.tile([1, 128], fp32, name="ones_sb" + sx)
        id_sb, f_id = tc.tile([128, 128], fp32, name="id_sb" + sx)
        tvs_sb, f_tvs = tc.tile([128, 16], fp32, name="tvs_sb" + sx)
        for sb, dr in ((xb_sb, xb_d), (wstat_sb, wstat_d), (wkt_sb, wkt_d),
                       (wqt_sb, wqt_d), (bkt_sb, bkt_d), (bqt_sb, bqt_d),
                       (ones_sb, ones_d), (id_sb, id_d), (tvs_sb, tvs_d)):
            nc.gpsimd.dma_start(sb[:], dr[:, :])

        # main PSUM pool used across all phases
        p512_cm = tc.tile_pool(name="p512" + sx, bufs=2, space="PSUM")
        p512 = p512_cm.__enter__()

        # ---- persistent big tensors ----
        # Alloc order is load-bearing: the tile allocator releases strictly
        # LIFO, so tiles that die earliest are allocated last.
        QT, f_QT = tc.tile([128, 2, N], fp32, name="QT" + sx)
        if OUT_MODE == "tri":
            KpT, f_KpT = tc.tile([128, 2, N], fp32, name="KpT" + sx)
            iotab, f_iotab = tc.tile([128, N], fp32, name="iotab" + sx)
            iota_sb, f_iota = tc.tile([1, N], fp32, name="iota_sb" + sx)
            keysN, f_keysN = tc.tile([128, 16, 256], fp32, name="keysN" + sx)
        keysT, f_keysT = tc.tile([128, 2, N], fp32, name="keysT" + sx)

        # ---- prologue: node -> nodeT -> keysT (+ keysN for tri) ----
        nodeN, f_nodeN = tc.tile([128, 16, 256], fp32, name="nodeN" + sx)
        nodeT, f_nodeT = tc.tile([128, 2, N], fp32, name="nodeT" + sx)
        for c in range(16):
            src = node_a if c < 8 else node_b
            c0_ = c if c < 8 else c - 8
            nc.gpsimd.dma_start(nodeN[:, c, :], src[c0_ * 128:(c0_ + 1) * 128, :])
        ptp_cm = tc.tile_pool(name="ptp" + sx, bufs=2, space="PSUM")
        ptp = ptp_cm.__enter__()
        for c in range(16):
            for k in range(2):
                pt = ptp.tile([128, 128], fp32)
                nc.tensor.transpose(pt[:], nodeN[:, c, k * 128:(k + 1) * 128], id_sb[:])
                nc.scalar.activation(nodeT[:, k, c * 128:(c + 1) * 128], pt[:], Act.Copy)
        ptp_cm.__exit__(None, None, None)
        for j2 in range(2):
            for nb in range(4):
                pk = p512.tile([128, 512], fp32, tag="ps")
                for k in range(2):
                    nc.tensor.matmul(pk[:], wkt_sb[:, (k * 2 + j2) * 128:(k * 2 + j2 + 1) * 128],
                                     nodeT[:, k, nb * 512:(nb + 1) * 512],
                                     start=(k == 0), stop=(k == 1))
                nc.vector.tensor_scalar(out=keysT[:, j2, nb * 512:(nb + 1) * 512],
                                        in0=pk[:], scalar1=bkt_sb[:, j2:j2 + 1],
                                        scalar2=None, op0=Alu.add)
        if OUT_MODE == "tri":
            # keysN[n(p), m(f)] = (node @ Wk^T), unbiased, n-on-partition.
            # Used later to gather keys by selection order via a one-hot matmul.
            for c in range(16):
                pn = p512.tile([128, 256], fp32, tag="ps")
                for kc in range(2):
                    nc.tensor.matmul(pn[:], nodeT[:, kc, c * 128:(c + 1) * 128],
                                     wkt_sb[:, kc * 256:(kc + 1) * 256],
                                     start=(kc == 0), stop=(kc == 1))
                nc.scalar.activation(keysN[:, c, :], pn[:], Act.Copy)
        f_nodeT()
        f_nodeN()

        # ---- phase 1: LSTM unroll -> HTx ----
        HTx, f_HTx = tc.tile([128, 2, N + 1], fp32, name="HTx" + sx)
        ct, f_ct = tc.tile([128, 2], fp32, name="ct" + sx)
        gsb, f_gsb = tc.tile([128, 8], fp32, name="gsb" + sx)
        sfo, f_sfo = tc.tile([128, 6], fp32, name="sfo" + sx)
        gt, f_gt = tc.tile([128, 2], fp32, name="gt" + sx)
        t1, f_t1 = tc.tile([128, 2], fp32, name="t1" + sx)
        tct, f_tct = tc.tile([128, 2], fp32, name="tct" + sx)
        nc.gpsimd.dma_start(HTx[:, 0, 0:1], h0_d[:, 0:1])
        nc.gpsimd.dma_start(HTx[:, 1, 0:1], h0_d[:, 1:2])
        nc.gpsimd.dma_start(ct[:], c0_d[:, :])
        gpp_cm = tc.tile_pool(name="gpp" + sx, bufs=2, space="PSUM")
        gpp = gpp_cm.__enter__()
        with tc.For_i(0, N, step=U) as iv:
            for u in range(U):
                t = iv + u
                gp = gpp.tile([128, 8], fp32)
                for m2 in range(8):
                    nc.tensor.matmul(gp[:, m2:m2 + 1],
                                     wstat_sb[:, (m2 * 2) * 128:(m2 * 2 + 1) * 128],
                                     HTx[:, 0, ds(t, 1)], start=True, stop=False)
                    nc.tensor.matmul(gp[:, m2:m2 + 1],
                                     wstat_sb[:, (m2 * 2 + 1) * 128:(m2 * 2 + 2) * 128],
                                     HTx[:, 1, ds(t, 1)], start=False, stop=True)
                nc.vector.tensor_tensor(out=gsb[:], in0=gp[:], in1=xb_sb[:], op=Alu.add)
                nc.scalar.activation(sfo[:], gsb[:, 0:6], Act.Sigmoid)
                nc.scalar.activation(gt[:], gsb[:, 6:8], Act.Tanh)
                nc.vector.tensor_tensor(out=t1[:], in0=sfo[:, 0:2], in1=gt[:], op=Alu.mult)
                nc.vector.tensor_tensor(out=ct[:], in0=sfo[:, 2:4], in1=ct[:], op=Alu.mult)
                nc.vector.tensor_tensor(out=ct[:], in0=ct[:], in1=t1[:], op=Alu.add)
                nc.scalar.activation(tct[:], ct[:], Act.Tanh)
                nc.vector.tensor_tensor(out=HTx[:, 0, ds(t + 1, 1)],
                                        in0=sfo[:, 4:5], in1=tct[:, 0:1], op=Alu.mult)
                nc.vector.tensor_tensor(out=HTx[:, 1, ds(t + 1, 1)],
                                        in0=sfo[:, 5:6], in1=tct[:, 1:2], op=Alu.mult)
        gpp_cm.__exit__(None, None, None)

        # ---- QT = Wq @ h + bq (feature-on-partition) ----
        for j2 in range(2):
            for tb in range(4):
                pq = p512.tile([128, 512], fp32, tag="ps")
                for k in range(2):
                    nc.tensor.matmul(pq[:], wqt_sb[:, (k * 2 + j2) * 128:(k * 2 + j2 + 1) * 128],
                                     HTx[:, k, 1 + tb * 512:1 + (tb + 1) * 512],
                                     start=(k == 0), stop=(k == 1))
                nc.vector.tensor_scalar(out=QT[:, j2, tb * 512:(tb + 1) * 512],
                                        in0=pq[:], scalar1=bqt_sb[:, j2:j2 + 1],
                                        scalar2=None, op0=Alu.add)
        f_tct(); f_t1(); f_gt(); f_sfo(); f_gsb(); f_ct(); f_HTx()

        # ---- phase 2+3 interleaved: ST blocks + argmax-rank chain ----
        if OUT_MODE != "tri":
            rb, f_maskb = tc.tile([128, N], fp32, name="rb" + sx)
            trs, f_trs = tc.tile([16, 128], fp32, name="trs" + sx)
        rankn, f_rankn = tc.tile([128, 16], fp32, name="rankn" + sx)
        if OUT_MODE != "tri":
            rr, f_rr = tc.tile([1, N], fp32, name="rr" + sx)
        stp_cm = tc.tile_pool(name="stp" + sx, bufs=2)
        stp = stp_cm.__enter__()
        ma, f_ma = tc.tile([128, 16], fp32, name="ma" + sx)
        ms, f_ms = tc.tile([128, 16], fp32, name="ms" + sx)
        mk, f_mk = tc.tile([128, 16], fp32, name="mk" + sx)
        pm, f_pm = tc.tile([128, 1], fp32, name="pm" + sx)
        gm, f_gm = tc.tile([1, 1], fp32, name="gm" + sx)
        dl, f_dl = tc.tile([128, 16], fp32, name="dl" + sx)
        tpp_cm = tc.tile_pool(name="tpp" + sx, bufs=2, space="PSUM")
        tpp = tpp_cm.__enter__()
        gbp_cm = tc.tile_pool(name="gbp" + sx, bufs=2, space="PSUM")
        gbp = gbp_cm.__enter__()
        nc.vector.memset(ma[:], 0.0)
        nc.vector.memset(ms[:], 0.0)

        def emit_st_block(tb):
            st_tb = stp.tile([128, 16, 512], fp32, name=f"st{tb}" + sx, tag="st")
            for c in range(16):
                pS = p512.tile([128, 512], fp32, tag="ps")
                for k in range(2):
                    nc.tensor.matmul(pS[:], keysT[:, k, c * 128:(c + 1) * 128],
                                     QT[:, k, tb * 512:(tb + 1) * 512],
                                     start=(k == 0), stop=(k == 1))
                nc.scalar.activation(st_tb[:, c, :], pS[:], Act.Copy)
            return st_tb

        def emit_l3(st_tb):
            with tc.For_i(0, 512, step=U) as iv:
                for u in range(U):
                    tl_ = iv + u
                    nc.vector.tensor_tensor(out=mk[:], in0=st_tb[:, :, ds(tl_, 1)],
                                            in1=ma[:], op=Alu.add)
                    # ms += 1 for already-selected boxes (exact small ints)
                    nc.vector.scalar_tensor_tensor(out=ms[:], in0=ma[:],
                                                   scalar=-(2.0 ** -30),
                                                   in1=ms[:], op0=Alu.mult,
                                                   op1=Alu.add)
                    nc.vector.reduce_max(out=pm[:], in_=mk[:], axis=AX.X)
                    tp = tpp.tile([1, 128], fp32, tag="tp")
                    nc.tensor.transpose(tp[:], pm[:], id_sb[:])
                    nc.vector.reduce_max(out=gm[:], in_=tp[:], axis=AX.X)
                    gb = gbp.tile([128, 1], fp32)
                    nc.tensor.matmul(gb[:], ones_sb[:], gm[:], start=True, stop=True)
                    nc.vector.tensor_scalar(out=dl[:], in0=mk[:], scalar1=gb[:],
                                            scalar2=-(2.0 ** 30), op0=Alu.is_ge,
                                            op1=Alu.mult)
                    nc.vector.tensor_tensor(out=ma[:], in0=ma[:], in1=dl[:], op=Alu.add)

        blocks = [emit_st_block(0), emit_st_block(1)]
        emit_l3(blocks[0])
        blocks.append(emit_st_block(2))
        emit_l3(blocks[1])
        blocks.append(emit_st_block(3))
        emit_l3(blocks[2])
        emit_l3(blocks[3])

        # rank_n = 2047 - ms_n (exact small integers)
        nc.vector.tensor_scalar(out=rankn[:], in0=ms[:], scalar1=-1.0,
                                scalar2=2047.0, op0=Alu.mult, op1=Alu.add)

        if OUT_MODE == "tri":
            nc.gpsimd.dma_start(outr_d[:, :], rankn[:])
            # iotab[p, r] = r (broadcast of the column-index row)
            nc.gpsimd.dma_start(iota_sb[:], iota1_d[:, :])
            for g in range(4):
                pr = p512.tile([128, 512], fp32, tag="ps")
                nc.tensor.matmul(pr[:], ones_sb[:], iota_sb[0:1, g * 512:(g + 1) * 512],
                                 start=True, stop=True)
                nc.scalar.activation(iotab[:, g * 512:(g + 1) * 512], pr[:], Act.Copy)
            # K_permT[m, r] = keys[pi(r), m] + bk[m] via one-hot gather matmul:
            # Pi[n, r] = 1{rank[n] == r}; one-hot fp32 matmul copies values exactly.
            pip_cm = tc.tile_pool(name="pip" + sx, bufs=3)
            pip = pip_cm.__enter__()
            for j2 in range(2):
                for rg in range(4):
                    pq = p512.tile([128, 512], fp32, tag="ps")
                    for c in range(16):
                        pi = pip.tile([128, 512], fp32, tag="pi")
                        nc.vector.tensor_scalar(out=pi[:],
                                                in0=iotab[:, rg * 512:(rg + 1) * 512],
                                                scalar1=rankn[:, c:c + 1],
                                                scalar2=None, op0=Alu.is_equal)
                        nc.tensor.matmul(pq[:], keysN[:, c, j2 * 128:(j2 + 1) * 128],
                                         pi[:], start=(c == 0), stop=(c == 15))
                    nc.vector.tensor_scalar(out=KpT[:, j2, rg * 512:(rg + 1) * 512],
                                            in0=pq[:], scalar1=bkt_sb[:, j2:j2 + 1],
                                            scalar2=None, op0=Alu.add)
            pip_cm.__exit__(None, None, None)
            keysP4 = KpT
        else:
            # broadcast rank over rows: rb[p, n] = rank[n]
            tp2 = tpp.tile([16, 128], fp32, tag="tp")
            nc.tensor.transpose(tp2[:], rankn[:], id_sb[:])
            nc.scalar.activation(trs[:], tp2[:], Act.Copy)
            for c in range(16):
                nc.gpsimd.dma_start(rr[0:1, c * 128:(c + 1) * 128], trs[c:c + 1, :])
            for g in range(4):
                pr = p512.tile([128, 512], fp32, tag="ps")
                nc.tensor.matmul(pr[:], ones_sb[:], rr[0:1, g * 512:(g + 1) * 512],
                                 start=True, stop=True)
                nc.scalar.activation(rb[:, g * 512:(g + 1) * 512], pr[:], Act.Copy)
            iotab = rb
            keysP4 = keysT

        gbp_cm.__exit__(None, None, None)
        tpp_cm.__exit__(None, None, None)
        f_dl(); f_gm(); f_pm(); f_mk(); f_ms(); f_ma()
        stp_cm.__exit__(None, None, None)
        if OUT_MODE == "tri":
            f_rankn()
            f_keysT()
            f_keysN()
            f_iota()

        # ---- phase 4: probs rows, masked softmax, DMA out ----
        # mask = (iotab >= t): original column order compares rank[n] >= t;
        # permuted order compares column index r >= t (static triangle).
        rs4, f_rs4 = tc.tile([128, 4], fp32, name="rs4" + sx)
        rsum, f_rsum = tc.tile([128, 1], fp32, name="rsum" + sx)
        rinv, f_rinv = tc.tile([128, 1], fp32, name="rinv" + sx)
        rsc, f_rsc = tc.tile([128, 1], fp32, name="rsc" + sx)
        esp_cm = tc.tile_pool(name="esp" + sx, bufs=2)
        esp = esp_cm.__enter__()
        eop_cm = tc.tile_pool(name="eop" + sx, bufs=2)
        eop = eop_cm.__enter__()
        for blk in range(16):
            es = esp.tile([128, N], fp32, name=f"es{blk}" + sx, tag="es")
            for nb in range(4):
                pS = p512.tile([128, 512], fp32, tag="ps")
                for k in range(2):
                    nc.tensor.matmul(pS[:], QT[:, k, blk * 128:(blk + 1) * 128],
                                     keysP4[:, k, nb * 512:(nb + 1) * 512],
                                     start=(k == 0), stop=(k == 1))
                nc.scalar.activation(es[:, nb * 512:(nb + 1) * 512], pS[:], Act.Exp)
                nc.vector.scalar_tensor_tensor(
                    out=es[:, nb * 512:(nb + 1) * 512],
                    in0=iotab[:, nb * 512:(nb + 1) * 512],
                    scalar=tvs_sb[:, blk:blk + 1],
                    in1=es[:, nb * 512:(nb + 1) * 512],
                    op0=Alu.is_ge, op1=Alu.mult,
                    accum_out=rs4[:, nb:nb + 1])
            nc.vector.reduce_sum(out=rsum[:], in_=rs4[:], axis=AX.X)
            nc.vector.reciprocal(rinv[:], rsum[:])
            eo = eop.tile([128, N], odt, name=f"eo{blk}" + sx, tag="eo")
            if OUT_MODE in ("tri", "u8"):
                nc.vector.tensor_scalar(out=rsc[:], in0=rinv[:], scalar1=OUT_SCALE,
                                        scalar2=None, op0=Alu.mult)
                for nb in range(4):
                    nc.scalar.activation(eo[:, nb * 512:(nb + 1) * 512],
                                         es[:, nb * 512:(nb + 1) * 512],
                                         Act.Copy, scale=rsc[:])
            else:
                for nb in range(4):
                    nc.scalar.activation(eo[:, nb * 512:(nb + 1) * 512],
                                         es[:, nb * 512:(nb + 1) * 512],
                                         Act.Copy, scale=rinv[:])
            if OUT_MODE == "tri":
                for sub in range(128 // RB):
                    i = blk * (128 // RB) + sub  # global band index
                    nc.gpsimd.dma_start(outp_d[:, CO[i]:CO[i] + BW[i]],
                                        eo[sub * RB:(sub + 1) * RB, i * RB:N])
            else:
                nc.gpsimd.dma_start(out_d[blk * 128:(blk + 1) * 128, :], eo[:])
        eop_cm.__exit__(None, None, None)
        esp_cm.__exit__(None, None, None)
        f_rsc(); f_rinv(); f_rsum(); f_rs4()
        if OUT_MODE == "tri":
            f_iotab(); f_KpT(); f_QT()
        else:
            f_rr(); f_rankn(); f_trs(); f_maskb()
            f_keysT(); f_QT()
        p512_cm.__exit__(None, None, None)
        f_tvs(); f_id(); f_ones(); f_bqt(); f_bkt(); f_wqt(); f_wkt(); f_wstat(); f_xb()

    from concourse import tile
    with tile.TileContext(nc) as tc:
        emit_pass(tc, 0)

    nc.compile()
    _cache["nc"] = nc
    return nc


def _prep_weights(inputs):
    """Per-core-identical operands, keyed by dram tensor name (per-core shapes)."""
    f32 = np.float32
    decoder_init = np.asarray(inputs["decoder_init"], dtype=f32)
    hidden0 = np.asarray(inputs["hidden0"], dtype=f32)
    w_ih = np.asarray(inputs["w_ih"], dtype=f32)
    w_hh = np.asarray(inputs["w_hh"], dtype=f32)
    b_ih = np.asarray(inputs["b_ih"], dtype=f32)
    b_hh = np.asarray(inputs["b_hh"], dtype=f32)
    Wq = np.asarray(inputs["Wq"], dtype=f32)
    bq = np.asarray(inputs["bq"], dtype=f32)
    Wk = np.asarray(inputs["Wk"], dtype=f32)
    bk = np.asarray(inputs["bk"], dtype=f32)

    perm = np.concatenate([np.arange(0, 256), np.arange(256, 512),
                           np.arange(768, 1024), np.arange(512, 768)])
    w_hh_p = w_hh[perm]
    x_proj = decoder_init @ w_ih.T + b_ih
    xb = np.ascontiguousarray(((x_proj + b_hh)[perm]).reshape(8, 128).T, dtype=f32)
    wstat = np.zeros((128, 2048), f32)
    for m2 in range(8):
        for k in range(2):
            blockT = w_hh_p[m2 * 128:(m2 + 1) * 128, k * 128:(k + 1) * 128].T
            wstat[:, (m2 * 2 + k) * 128:(m2 * 2 + k + 1) * 128] = blockT
    WkT = Wk.T
    WqT = Wq.T
    wkt = np.zeros((128, 512), f32)
    wqt = np.zeros((128, 512), f32)
    for k in range(2):
        for j in range(2):
            wkt[:, (k * 2 + j) * 128:(k * 2 + j + 1) * 128] = \
                WkT[k * 128:(k + 1) * 128, j * 128:(j + 1) * 128]
            wqt[:, (k * 2 + j) * 128:(k * 2 + j + 1) * 128] = \
                WqT[k * 128:(k + 1) * 128, j * 128:(j + 1) * 128]
    bkt = np.ascontiguousarray(bk.reshape(2, 128).T, dtype=f32)
    bqt = np.ascontiguousarray(bq.reshape(2, 128).T, dtype=f32)
    h0c = np.ascontiguousarray(hidden0.reshape(2, 128).T, dtype=f32)
    ones1 = np.ones((1, 128), f32)
    ident = np.eye(128, dtype=f32)
    tvs = (np.arange(128, dtype=f32)[:, None] +
           128.0 * np.arange(16, dtype=f32)[None, :]).astype(f32)
    w = dict(xb=xb, wstat=wstat, wkt=wkt, wqt=wqt, bkt=bkt, bqt=bqt,
             ones1=ones1, ident=ident, tvs=tvs, h0=h0c)
    if OUT_MODE == "tri":
        w["iota1"] = np.arange(N, dtype=f32)[None, :]
    return w


def _weights_key(inputs):
    h = hashlib.blake2b(digest_size=16)
    for k in ("decoder_init", "hidden0", "w_ih", "w_hh", "b_ih", "b_hh",
              "Wq", "bq", "Wk", "bk"):
        a = np.ascontiguousarray(np.asarray(inputs[k], dtype=np.float32))
        h.update(a.tobytes())
    return h.digest()


def _get_rt():
    if "rt" in _cache:
        return _cache["rt"]
    nc = _build()
    import concourse.mybir as mybir
    from concourse.bass2jax import (_bass_exec_p, install_neuronx_cc_hook,
                                    partition_id_tensor)
    import jax
    import jax.numpy as jnp
    from jax.sharding import Mesh, PartitionSpec, NamedSharding
    from jax.experimental.shard_map import shard_map

    install_neuronx_cc_hook()
    partition_name = nc.partition_id_tensor.name if nc.partition_id_tensor else None

    in_names = []
    out_names = []
    out_avals = []
    for alloc in nc.m.functions[0].allocations:
        if not isinstance(alloc, mybir.MemoryLocationSet):
            continue
        name = alloc.memorylocations[0].name
        if alloc.kind == "ExternalInput":
            if name != partition_name:
                in_names.append(name)
        elif alloc.kind == "ExternalOutput":
            out_names.append(name)
            out_avals.append(jax.core.ShapedArray(tuple(alloc.tensor_shape),
                                                  mybir.dt.np(alloc.dtype)))
    n_params = len(in_names)
    n_outs = len(out_avals)
    in_names_full = list(in_names) + out_names
    if partition_name is not None:
        in_names_full.append(partition_name)
    donate = tuple(range(n_params, n_params + n_outs))

    def _body(*args):
        operands = list(args)
        if partition_name is not None:
            operands.append(partition_id_tensor())
        return tuple(_bass_exec_p.bind(
            *operands, out_avals=tuple(out_avals), in_names=tuple(in_names_full),
            out_names=tuple(out_names), lowering_input_output_aliases=(),
            sim_require_finite=True, sim_require_nnan=True, nc=nc))

    devices = jax.devices()[:NCORES]
    gsz = NCORES // GROUPS
    groups = []
    in_specs = (PartitionSpec("core"),) * (n_params + n_outs)
    out_specs = (PartitionSpec("core"),) * n_outs
    for g in range(GROUPS):
        devs = devices[g * gsz:(g + 1) * gsz]
        mesh = Mesh(np.asarray(devs), ("core",))
        sh = NamedSharding(mesh, PartitionSpec("core"))
        sharded = jax.jit(
            shard_map(_body, mesh=mesh, in_specs=in_specs, out_specs=out_specs,
                      check_rep=False),
            donate_argnums=donate, keep_unused=True)
        zeros_jit = jax.jit(
            lambda gsz=gsz: tuple(
                jnp.zeros((gsz * av.shape[0], *av.shape[1:]), av.dtype)
                for av in out_avals),
            out_shardings=(sh,) * n_outs)
        groups.append(SimpleNamespace(devs=devs, sh=sh, sharded=sharded,
                                      zeros_jit=zeros_jit))

    rt = SimpleNamespace(nc=nc, jax=jax, in_names=in_names, out_names=out_names,
                         groups=groups, gsz=gsz,
                         weights_dev=None, weights_key=None,
                         pool=ThreadPoolExecutor(max_workers=32),
                         up_pool=ThreadPoolExecutor(max_workers=2 * gsz))
    _cache["rt"] = rt
    return rt


def _upload_weights(rt, inputs):
    key = _weights_key(inputs)
    if rt.weights_key == key and rt.weights_dev is not None:
        return rt.weights_dev
    w = _prep_weights(inputs)
    dev = []
    for grp in rt.groups:
        gd = {}
        for name, arr in w.items():
            t = np.tile(arr, (rt.gsz, 1))
            gd[name] = rt.jax.device_put(t, grp.sh)
        dev.append(gd)
    for gd in dev:
        rt.jax.block_until_ready(list(gd.values()))
    rt.weights_dev = dev
    rt.weights_key = key
    return dev


def _run(inputs, trace=False, tmpdir=None):
    rt = _get_rt()
    jax = rt.jax
    f32 = np.float32
    gsz = rt.gsz

    weights = _upload_weights(rt, inputs)

    node_embedding = np.asarray(inputs["node_embedding"], dtype=f32)
    z_g = np.asarray(inputs["z_g"], dtype=f32)
    c0_all = np.ascontiguousarray(
        z_g.reshape(NCORES, 2, 128).transpose(0, 2, 1)).reshape(NCORES * 128, 2)

    H = N // 2
    res = np.empty((B, N, N), f32)
    inv_scale = f32(1.0 / OUT_SCALE)
    recon_futs = []
    prev_nodes = None

    # All groups' upload tasks are queued up front on a worker pool sized to
    # one group's chunk count: group g+1's transfers begin exactly as group
    # g's workers free up (backpressure stagger, no sync RPC). The tasks
    # block until their bytes land so pool slots model in-flight transfers.
    def _up_block(b, j, grp):
        a = jax.device_put(node_embedding[b, j * H:(j + 1) * H],
                           grp.devs[b % gsz])
        jax.block_until_ready(a)
        return a

    if GROUPS > 1:
        up_pool = rt.up_pool
        all_futs = {(b, j): up_pool.submit(_up_block, b, j, rt.groups[b // gsz])
                    for b in range(NCORES) for j in range(2)}

    for g, grp in enumerate(rt.groups):
        cores = list(range(g * gsz, (g + 1) * gsz))
        # per-call activations upload, shard-parallel (node split into two
        # tensors so 2*gsz transfers run concurrently)
        if GROUPS > 1:
            futs = all_futs
        else:
            futs = {(b, j): rt.pool.submit(
                        jax.device_put, node_embedding[b, j * H:(j + 1) * H],
                        grp.devs[b - g * gsz])
                    for b in cores for j in range(2)}
        zeros = grp.zeros_jit()
        node_arrs = [
            jax.make_array_from_single_device_arrays(
                (gsz * H, M), grp.sh, [futs[(b, j)].result() for b in cores])
            for j in range(2)]
        c0_g = c0_all[g * gsz * 128:(g + 1) * gsz * 128]
        per_call = {"node_a": node_arrs[0], "node_b": node_arrs[1], "c0": c0_g}
        args = [per_call[nm] if nm in per_call else weights[g][nm]
                for nm in rt.in_names]
        outs = grp.sharded(*args, *zeros)
        out_by_name = dict(zip(rt.out_names, outs))

        if OUT_MODE == "tri":
            ranks_fut = rt.pool.submit(np.asarray, out_by_name["outr"])

            def _recon(s, g=g, ranks_fut=ranks_fut):
                bl = s.index[0].start // RB
                buf = np.asarray(s.data)  # [RB, CW] u8
                rankn = ranks_fut.result()[bl * 128:(bl + 1) * 128]  # [128, 16]
                rank = rankn.T.reshape(N).astype(np.int64)  # rank[n]
                P = np.zeros((N, N), np.uint8)
                for i in range(NB):
                    P[i * RB:(i + 1) * RB, i * RB:] = buf[:, CO[i]:CO[i] + BW[i]]
                a = np.take(P, rank, axis=1)
                np.multiply(a, inv_scale, out=res[g * gsz + bl])

            recon_futs += [rt.pool.submit(_recon, s)
                           for s in out_by_name["outp"].addressable_shards]
        else:
            out_arr = out_by_name["out"]

            def _fetch(task, g=g):
                s, j = task
                bl = s.index[0].start // N
                a = np.asarray(s.data[j * H:(j + 1) * H])
                dst = res[g * gsz + bl, j * H:(j + 1) * H]
                if OUT_MODE == "u8":
                    np.multiply(a, inv_scale, out=dst)
                else:
                    np.copyto(dst, a, casting="unsafe")

            recon_futs += [rt.pool.submit(_fetch, (s, j))
                           for s in out_arr.addressable_shards for j in range(2)]
        prev_nodes = node_arrs

    for f in recon_futs:
        f.result()
    return res, SimpleNamespace(exec_time_ns=None, results=None)


def kernel(**inputs) -> np.ndarray:
    out, _ = _run(inputs, trace=False)
    return out
